# revision 5
# baseline (speedup 1.0000x reference)
"""12-block transformer encoder (B=2, S=2048, D=512, H=8, DHID=1024) on 8 trn2 cores.

Sequence-parallel: core c owns batch c//4, tokens 512*(c%4)..+512. Weights
replicated, fp8e4m3 at 64x scale (escapes e4m3 denormals; folds undone in
existing scalar slots). Residual stream f32 at 64x. Projections + attn@V run
fp8 DoubleRow (2 contraction chunks per matmul, 0.5 cyc/row). Scores fp8
1 cyc/row. Softmax exp split across engines: exact Exp on Activation, bit-trick
fast-exp (tensor_scalar -> uint8 bits viewed as fp8e4) on Pool and DVE,
interleaved per score-group so all three engines run concurrently.
LayerNorm rstd = Exp(-0.5*Ln(D*var)+lnC) on Activation; a get_activation_tables
patch during build makes the table pass pick the single set containing
exp+ln+relu, so there is exactly one table load. LN output biases are folded
into the next layer's biases host-side. K^T and aug-V AllGathered in fp8 as
two collectives (K first so scores start sooner; ones columns ride with V so
the softmax denominator falls out of the attn@V matmul).
"""
import sys
import numpy as np

for _p in ("/opt/trn_rl_repo", "/root/.axon_site/_ro/trn_rl_repo"):
    if _p not in sys.path:
        sys.path.insert(0, _p)

P = 128
B, S, D = 2, 2048, 512
H, DH, DHID = 8, 64, 1024
NB = 12
TOK = 512            # tokens per core
KT = D // P          # 4 contraction chunks over D
N_CORES = 8
GROUPS = [[0, 1, 2, 3], [4, 5, 6, 7]]
VW = 768             # aug-v row width per token ptile (4 pairs x 192)
WS = 64.0            # weight scale
LOG2E = 1.4426950408889634
SPLIT_AG = True

_CACHE = {}


def _build(nb, reps=1):
    import os
    import concourse.bass as bass
    import concourse.mybir as mybir
    import concourse.tile as tile
    from concourse import bacc
    import concourse.hw_specs as hw_specs
    import concourse.bacc as bacc_mod

    f32 = mybir.dt.float32
    f16 = mybir.dt.float16
    f8 = mybir.dt.float8e4
    u8 = mybir.dt.uint8
    AF = mybir.ActivationFunctionType
    OP = mybir.AluOpType
    DR = mybir.MatmulPerfMode.DoubleRow

    # Steer the act-table pass to the one set holding exp+ln+relu so the
    # kernel needs a single table load. Runtime-correct: we only narrow the
    # claimed contents of the other sets; ids stay the json indices.
    _orig_tables = hw_specs.get_activation_tables

    def _patched(arch):
        t = _orig_tables(arch)
        for name, s in t.items():
            if name != "natural_log_exp_and_others":
                s.discard(AF.Exp)
                s.discard(AF.Ln)
                s.discard(AF.Relu)
        return t

    sim1 = os.environ.get("KSIM") in ("1", "2")
    sim_cheap = os.environ.get("KSIM") == "2"
    nc = bacc.Bacc("TRN2", target_bir_lowering=False, debug=False,
                   num_devices=(1 if sim1 else N_CORES))

    xT_in = nc.declare_dram_parameter("xT", [D, TOK], f32, isOutput=False)
    wq_in = nc.declare_dram_parameter("wq", [nb, D, D], f8, isOutput=False)
    wk_in = nc.declare_dram_parameter("wk", [nb, D, D], f8, isOutput=False)
    wv_in = nc.declare_dram_parameter("wv", [nb, D, D], f8, isOutput=False)
    fc_in = nc.declare_dram_parameter("fcw", [nb, D, D], f8, isOutput=False)
    w1_in = nc.declare_dram_parameter("w1", [nb, D, DHID], f16, isOutput=False)
    w2_in = nc.declare_dram_parameter("w2", [nb, DHID, D], f16, isOutput=False)
    bias_in = nc.declare_dram_parameter("biases", [nb, P, 44], f32,
                                        isOutput=False)
    yT_out = nc.declare_dram_parameter("yT", [D, TOK], f32, isOutput=True)

    inv_sqrt_d = float(1.0 / np.sqrt(D))
    SEXP = inv_sqrt_d / (WS * WS)     # exp arg = psum * SEXP
    FE_A = LOG2E * 8.0 * SEXP         # fast-exp bits = psum*FE_A + FE_B
    FE_B = 7.0 * 8.0 - 0.58
    BOF = {"bq": 0, "bk": 4, "fcb": 8, "b2": 12, "g1": 16, "be1": 20,
           "g2": 24, "be2": 28, "b1": 32, "be2s": 40}

    with tile.TileContext(nc) as tc:
        with tc.tile_pool(name="w", bufs=2) as sbw, \
             tc.tile_pool(name="bias", bufs=2) as sbb, \
             tc.tile_pool(name="act", bufs=1) as sba, \
             tc.tile_pool(name="e", bufs=8) as sbe, \
             tc.tile_pool(name="pers", bufs=1) as sbp, \
             tc.tile_pool(name="psS", bufs=2, space="PSUM") as psS, \
             tc.tile_pool(name="psA", bufs=4, space="PSUM") as psA, \
             tc.tile_pool(name="dram", bufs=2, space="DRAM") as dram:

            ones16 = sbp.tile([P, P], f16, tag="ones16", name="ones16")
            nc.vector.memset(ones16[:], 1.0)
            lnC = sbp.tile([P, 1], f32, tag="lnC", name="lnC")
            nc.vector.memset(lnC[:], float(np.log(np.sqrt(D) / WS)))
            ones8 = sbp.tile([P, 2 * P], f8, tag="ones8", name="ones8")
            nc.vector.memset(ones8[:], 1.0)
            on2 = ones8[:].rearrange("p (t m) -> p t m", t=2)
            ktf = [sbp.tile([P, S], f8, tag=f"ktf{p}", name=f"ktf{p}")
                   for p in range(4)]
            # all 16 key-ptile aug-v tiles in one tensor: [P, 16*768] fp8
            va_all = sbp.tile([P, 16 * VW], f8, tag="va", name="va")
            # own-token aug-v staging: [P, 4*768], ones written once
            v_own = sbp.tile([P, 4 * VW], f8, tag="vown", name="vown")
            for tt_ in (va_all, v_own):
                oc = tt_[:].rearrange("p (a b) -> p a b", b=64)
                nc.vector.memset(oc[:, 1:oc.shape[1]:3, :], 1.0)

            # residual stream (64x) + fp8 copy
            xin = [sba.tile([P, TOK], f32, tag=f"x32_{k}", name=f"x32_{k}")
                   for k in range(KT)]
            x8w = sba.tile([P, KT * TOK], f8, tag="x8w", name="x8w")
            for k in range(KT):
                nc.sync.dma_start(xin[k][:], xT_in[P * k:P * (k + 1), :])
                nc.vector.tensor_scalar_mul(
                    x8w[:, TOK * k:TOK * (k + 1)], xin[k][:], 1.0 / WS)

            def pair2(ap_wide, k, width, lo, hi):
                """[P, 2, hi-lo] view: chunks k,k+1 of ap_wide, cols lo:hi."""
                r = ap_wide.rearrange("p (k c) -> p k c", c=width)
                return r[:, k:k + 2, lo:hi]

            def layer_norm(x4, g_ap, ln8_out, out_extra):
                """x4: 4 f32 [P,TOK] tiles at 64x. Writes normalized*g (no
                +bias: folded downstream) as fp8 into ln8_out (wide); calls
                out_extra(k, u32, eng) for extra outputs (u32 f32)."""
                m8 = sba.tile([P, KT * TOK], f8, tag="ln_m8", name="ln_m8")
                sq8 = sba.tile([P, KT * TOK], f8, tag="ln_sq8", name="ln_sq8")
                for k in range(KT):
                    cs = slice(TOK * k, TOK * (k + 1))
                    eng = (nc.gpsimd, nc.vector)[k % 2]
                    eng.tensor_scalar_mul(m8[:, cs], x4[k][:], 1.0 / WS)
                    eng.tensor_mul(sq8[:, cs], m8[:, cs], m8[:, cs])
                psum_s = psA.tile([P, TOK], f32, tag="ps", name="ps")
                psum_q = psA.tile([P, TOK], f32, tag="ps", name="ps")
                for k in (0, 2):
                    nc.tensor.matmul(psum_s[:], on2,
                                     pair2(m8[:], k, TOK, 0, TOK),
                                     start=(k == 0), stop=(k == 2),
                                     perf_mode=DR)
                    nc.tensor.matmul(psum_q[:], on2,
                                     pair2(sq8[:], k, TOK, 0, TOK),
                                     start=(k == 0), stop=(k == 2),
                                     perf_mode=DR)
                a = sba.tile([P, TOK], f32, tag="ln_a", name="ln_a")
                nc.vector.tensor_scalar_mul(a[:], psum_s[:], WS / D)
                a2 = sba.tile([P, TOK], f32, tag="ln_a2", name="ln_a2")
                nc.gpsimd.tensor_mul(a2[:], a[:], a[:])
                pt = sba.tile([P, TOK], f32, tag="ln_pt", name="ln_pt")
                nc.vector.scalar_tensor_tensor(
                    pt[:], in0=a2[:], scalar=float(-D / (WS * WS)),
                    in1=psum_q[:], op0=OP.mult, op1=OP.add)
                # rstd*C = exp(-0.5*ln(pt) + ln(C)), C = sqrt(D)/WS
                lnv = sba.tile([P, TOK], f32, tag="ln_lnv", name="ln_lnv")
                nc.scalar.activation(lnv[:], pt[:], AF.Ln, bias=0.0, scale=1.0)
                rstd = sba.tile([P, TOK], f32, tag="ln_rstd", name="ln_rstd")
                nc.scalar.activation(rstd[:], lnv[:], AF.Exp,
                                     bias=lnC[:, 0:1], scale=-0.5)
                for k in range(KT):
                    ei = k % 2
                    eng = (nc.vector, nc.gpsimd)[ei]
                    t = sba.tile([P, TOK], f32, tag=f"ln_t{ei}",
                                 name=f"ln_t{ei}")
                    eng.tensor_sub(t[:], x4[k][:], a[:])

                    def stt_mm(out_ap, tin):
                        # (t * g) * rstd; Pool lacks scalar_tensor_tensor
                        if ei == 0:
                            eng.scalar_tensor_tensor(
                                out_ap, in0=tin, scalar=g_ap(k), in1=rstd[:],
                                op0=OP.mult, op1=OP.mult)
                        else:
                            w = sba.tile([P, TOK], f32, tag="ln_w1",
                                         name="ln_w1")
                            eng.tensor_scalar_mul(w[:], tin, g_ap(k))
                            eng.tensor_mul(out_ap, w[:], rstd[:])

                    if out_extra is None:
                        stt_mm(ln8_out[:, TOK * k:TOK * (k + 1)], t[:])
                    else:
                        u = sba.tile([P, TOK], f32, tag=f"ln_u{ei}",
                                     name=f"ln_u{ei}")
                        stt_mm(u[:], t[:])
                        eng.tensor_copy(ln8_out[:, TOK * k:TOK * (k + 1)],
                                        u[:])
                        out_extra(k, u, eng)

            for rep in range(reps):
              for l in range(nb):
                # ---- weights: one DMA per tensor (fp8) ----
                wq_t = sbw.tile([P, KT * D], f8, tag="wq", name="wq")
                wk_t = sbw.tile([P, KT * D], f8, tag="wk", name="wk")
                wv_t = sbw.tile([P, KT * D], f8, tag="wv", name="wv")
                fc_t = sbw.tile([P, KT * D], f8, tag="fcw", name="fcw")
                w1_t = sbw.tile([P, KT * DHID], f16, tag="w1", name="w1")
                w2_t = sbw.tile([P, 8 * D], f16, tag="w2", name="w2")
                for sb_t, src, width in ((wk_t, wk_in, D), (wv_t, wv_in, D),
                                         (wq_t, wq_in, D), (fc_t, fc_in, D),
                                         (w1_t, w1_in, DHID)):
                    nc.sync.dma_start(
                        sb_t[:].rearrange("p (k c) -> p k c", c=width),
                        src[l].rearrange("(k p) c -> p k c", p=P))
                nc.sync.dma_start(
                    w2_t[:].rearrange("p (k c) -> p k c", c=D),
                    w2_in[l].rearrange("(k p) c -> p k c", p=P))
                bt = sbb.tile([P, 44], f32, tag="bias", name="bias")
                nc.sync.dma_start(bt[:], bias_in[l, :, :])

                def bap(name, idx):
                    o = BOF[name] + idx
                    return bt[:, o:o + 1]

                # ---- k^T (feature-major, fp8 at 64x), AG_K ----
                kT8 = sba.tile([P, KT * TOK], f8, tag="kT8", name="kT8")
                for hp in range(4):
                    ps = psA.tile([P, TOK], f32, tag="ps", name="ps")
                    for k in (0, 2):
                        nc.tensor.matmul(
                            ps[:], pair2(wk_t[:], k, D, P * hp, P * (hp + 1)),
                            pair2(x8w[:], k, TOK, 0, TOK),
                            start=(k == 0), stop=(k == 2), perf_mode=DR)
                    nc.scalar.activation(
                        kT8[:, TOK * hp:TOK * (hp + 1)], ps[:], AF.Identity,
                        bias=bap("bk", hp), scale=1.0)
                cc_ik = dram.tile([D, TOK], f8, tag="cc_ik", name="cc_ik")
                cc_ok = dram.tile([4 * D, TOK], f8, tag="cc_ok", name="cc_ok")
                cc_iv = dram.tile([D, VW], f8, tag="cc_iv", name="cc_iv")
                cc_ov = dram.tile([4 * D, VW], f8, tag="cc_ov", name="cc_ov")
                nc.sync.dma_start(
                    cc_ik[:, :].rearrange("(hp p) c -> p hp c", p=P),
                    kT8[:].rearrange("p (hp c) -> p hp c", c=TOK))
                if sim1:
                    nc.sync.dma_start(cc_ok[0:D, :], cc_ik[:, :])
                    if not sim_cheap:
                        for r in range(1, 4):
                            nc.sync.dma_start(cc_ok[D * r:D * (r + 1), :],
                                              cc_ik[:, :])
                elif SPLIT_AG:
                    nc.gpsimd.collective_compute(
                        "AllGather", mybir.AluOpType.bypass,
                        replica_groups=GROUPS,
                        ins=[cc_ik[:].opt()], outs=[cc_ok[:].opt()])

                # ---- v (token-major, aug fp8 at 64x), AG_V ----
                for t in range(4):
                    ps = psA.tile([P, D], f32, tag="ps", name="ps")
                    for k in (0, 2):
                        nc.tensor.matmul(
                            ps[:], pair2(x8w[:], k, TOK, P * t, P * (t + 1)),
                            pair2(wv_t[:], k, D, 0, D),
                            start=(k == 0), stop=(k == 2), perf_mode=DR)
                    # one 4D-AP copy: heads (2p, 2p+1) -> aug slots (0, 2)
                    ps4 = ps[:].rearrange("p (pr w b) -> p pr w b", pr=4, w=2)
                    vo4 = v_own[:, VW * t:VW * (t + 1)].rearrange(
                        "p (pr w2 b) -> p pr w2 b", pr=4, w2=3)[:, :, 0:3:2, :]
                    nc.scalar.activation(vo4, ps4[:, :, :, :], AF.Copy,
                                         bias=0.0, scale=1.0)
                nc.sync.dma_start(
                    cc_iv[:, :].rearrange("(t p) c -> p t c", p=P),
                    v_own[:].rearrange("p (t c) -> p t c", c=VW))
                if sim1:
                    nc.sync.dma_start(cc_ov[0:D, :], cc_iv[:, :])
                    if not sim_cheap:
                        for r in range(1, 4):
                            nc.sync.dma_start(cc_ov[D * r:D * (r + 1), :],
                                              cc_iv[:, :])
                elif SPLIT_AG:
                    nc.gpsimd.collective_compute(
                        "AllGather", mybir.AluOpType.bypass,
                        replica_groups=GROUPS,
                        ins=[cc_iv[:].opt()], outs=[cc_ov[:].opt()])
                else:
                    nc.gpsimd.collective_compute(
                        "AllGather", mybir.AluOpType.bypass,
                        replica_groups=GROUPS,
                        ins=[cc_ik[:].opt(), cc_iv[:].opt()],
                        outs=[cc_ok[:].opt(), cc_ov[:].opt()])

                # ---- q^T fp8 (overlaps the collectives) ----
                qT8 = sba.tile([P, KT * TOK], f8, tag="qT8", name="qT8")
                for hp in range(4):
                    ps = psA.tile([P, TOK], f32, tag="ps", name="ps")
                    for k in (0, 2):
                        nc.tensor.matmul(
                            ps[:], pair2(wq_t[:], k, D, P * hp, P * (hp + 1)),
                            pair2(x8w[:], k, TOK, 0, TOK),
                            start=(k == 0), stop=(k == 2), perf_mode=DR)
                    nc.vector.tensor_scalar_add(
                        qT8[:, TOK * hp:TOK * (hp + 1)], ps[:], bap("bq", hp))

                # ---- gather-in loads ----
                for p in range(4):
                    nc.sync.dma_start(
                        ktf[p][:].rearrange("p (c w) -> p c w", w=TOK),
                        cc_ok[:, :].rearrange(
                            "(c p q) w -> c p q w", p=4, q=P)[:, p, :, :]
                        .rearrange("c q w -> q c w"))
                for cch in range(4):
                    nc.sync.dma_start(
                        va_all[:, VW * 4 * cch:VW * 4 * (cch + 1)]
                        .rearrange("p (j c) -> p j c", c=VW),
                        cc_ov[D * cch:D * (cch + 1), :]
                        .rearrange("(j p) c -> p j c", p=P))

                # ---- attention, head pairs ----
                oT8w = sba.tile([P, KT * TOK], f8, tag="oT8w", name="oT8w")
                va_r = va_all[:].rearrange("p (j c) -> p j c", c=VW)
                for hp in range(4):
                    qs = qT8[:, TOK * hp:TOK * (hp + 1)]
                    po_e = psA.tile([P, TOK], f32, tag="ps", name="ps")
                    po_o = psA.tile([P, TOK], f32, tag="ps", name="ps")
                    e_list = []
                    for g in range(8):
                        ps_e = psS.tile([P, 1024], f32, tag="ps_sc",
                                        name="ps_sc")
                        ps_o = psS.tile([P, 1024], f32, tag="ps_sc",
                                        name="ps_sc")
                        for c in range(2):
                            j = 2 * g + c
                            nc.tensor.matmul(
                                ps_e[:, TOK * c:TOK * (c + 1)],
                                ktf[hp][0:64, P * j:P * (j + 1)],
                                qs[0:64, :], start=True, stop=True)
                            nc.tensor.matmul(
                                ps_o[:, TOK * c:TOK * (c + 1)],
                                ktf[hp][64:128, P * j:P * (j + 1)],
                                qs[64:128, :], start=True, stop=True)
                        e_e = sbe.tile([P, 1024], u8, tag="e", name="e")
                        e_o = sbe.tile([P, 1024], u8, tag="e", name="e")
                        # interleave: Act every g; Pool/DVE alternate
                        nc.scalar.activation(e_e[:].bitcast(f8), ps_e[:],
                                             AF.Exp, bias=0.0, scale=SEXP)
                        if g == 0:
                            nc.scalar.activation(e_o[:].bitcast(f8), ps_o[:],
                                                 AF.Exp, bias=0.0, scale=SEXP)
                        else:
                            nc.vector.tensor_scalar(e_o[:], ps_o[:], FE_A,
                                                    FE_B, OP.mult, OP.add)
                        e_list.append((e_e, e_o))
                        if g >= 1:
                            pe_, po_ = e_list[g - 1]
                            jg = g - 1
                            nc.tensor.matmul(
                                po_e[:],
                                va_r[:, 2 * jg:2 * jg + 2,
                                     192 * hp:192 * hp + 128],
                                pe_[:].bitcast(f8).rearrange(
                                    "p (t c) -> p t c", t=2),
                                start=(jg == 0), stop=False, perf_mode=DR)
                            nc.tensor.matmul(
                                po_o[:],
                                va_r[:, 2 * jg:2 * jg + 2,
                                     192 * hp + 64:192 * hp + 192],
                                po_[:].bitcast(f8).rearrange(
                                    "p (t c) -> p t c", t=2),
                                start=(jg == 0), stop=False, perf_mode=DR)
                    pe_, po_ = e_list[7]
                    nc.tensor.matmul(
                        po_e[:], va_r[:, 14:16, 192 * hp:192 * hp + 128],
                        pe_[:].bitcast(f8).rearrange("p (t c) -> p t c", t=2),
                        start=False, stop=True, perf_mode=DR)
                    nc.tensor.matmul(
                        po_o[:], va_r[:, 14:16, 192 * hp + 64:192 * hp + 192],
                        po_[:].bitcast(f8).rearrange("p (t c) -> p t c", t=2),
                        start=False, stop=True, perf_mode=DR)
                    # normalize: denominators are rows dl; o rows ol
                    for par, po in ((0, po_e), (1, po_o)):
                        ol = slice(64 * par, 64 * par + 64)
                        dl = slice(64 * (1 - par), 64 * (1 - par) + 64)
                        dcp = sba.tile([P, TOK], f16, tag=f"dcp{par}",
                                       name=f"dcp{par}")
                        nc.scalar.activation(dcp[dl, :], po[dl, :], AF.Copy,
                                             bias=0.0, scale=1.0 / 64.0)
                        ps2 = psA.tile([P, TOK], f32, tag="ps", name="ps")
                        nc.tensor.matmul(ps2[:], ones16[dl, :], dcp[dl, :],
                                         start=True, stop=True)
                        rec = sba.tile([P, TOK], f32, tag=f"rec{par}",
                                       name=f"rec{par}")
                        nc.vector.reciprocal_approx_fast(rec[:], ps2[:])
                        nc.vector.scalar_tensor_tensor(
                            oT8w[:, TOK * hp:TOK * (hp + 1)][ol, :],
                            in0=po[ol, :], scalar=float(1.0 / WS),
                            in1=rec[ol, :], op0=OP.mult, op1=OP.mult)

                # ---- fc + residual (xmid = 64x) ----
                xmid = []
                for m in range(4):
                    ps = psA.tile([P, TOK], f32, tag="ps", name="ps")
                    for k in (0, 2):
                        nc.tensor.matmul(
                            ps[:], pair2(fc_t[:], k, D, P * m, P * (m + 1)),
                            pair2(oT8w[:], k, TOK, 0, TOK),
                            start=(k == 0), stop=(k == 2), perf_mode=DR)
                    xm = sba.tile([P, TOK], f32, tag=f"xmid{m}",
                                  name=f"xmid{m}")
                    nc.vector.scalar_tensor_tensor(
                        xm[:], in0=ps[:], scalar=bap("fcb", m), in1=xin[m][:],
                        op0=OP.add, op1=OP.add)
                    xmid.append(xm)

                # ---- LN1 -> MLP ----
                ln8w = sba.tile([P, KT * TOK], f16, tag="ln8w", name="ln8w")
                layer_norm(xmid, lambda k: bap("g1", k), ln8w, None)
                h8w = sba.tile([P, 8 * TOK], f16, tag="h8w", name="h8w")
                for m in range(8):
                    ps = psA.tile([P, TOK], f32, tag="ps", name="ps")
                    for k in range(KT):
                        nc.tensor.matmul(
                            ps[:],
                            w1_t[:, DHID * k + P * m:DHID * k + P * (m + 1)],
                            ln8w[:, TOK * k:TOK * (k + 1)],
                            start=(k == 0), stop=(k == KT - 1))
                    nc.scalar.activation(h8w[:, TOK * m:TOK * (m + 1)], ps[:],
                                         AF.Relu, bias=bap("b1", m), scale=1.0)
                xout = []
                for m in range(4):
                    ps = psA.tile([P, TOK], f32, tag="ps", name="ps")
                    for k in range(8):
                        nc.tensor.matmul(
                            ps[:], w2_t[:, D * k + P * m:D * k + P * (m + 1)],
                            h8w[:, TOK * k:TOK * (k + 1)],
                            start=(k == 0), stop=(k == 7))
                    tb = sba.tile([P, TOK], f32, tag=f"tb{m}", name=f"tb{m}")
                    nc.vector.tensor_scalar(tb[:], ps[:], 1.0 / WS,
                                            bap("b2", m), OP.mult, OP.add)
                    xo = sba.tile([P, TOK], f32, tag=f"xout{m}",
                                  name=f"xout{m}")
                    nc.gpsimd.tensor_add(xo[:], tb[:], xmid[m][:])
                    xout.append(xo)

                # ---- LN2 -> next block's x (fp8 sans bias + f32@64x) ----
                x8w = sba.tile([P, KT * TOK], f8, tag="x8w", name="x8w")
                xin = [sba.tile([P, TOK], f32, tag=f"x32_{k}",
                                name=f"x32_{k}") for k in range(KT)]
                last = (rep == reps - 1) and (l == nb - 1)

                if last:
                    def extra(k, u, eng, xin=xin):
                        eng.tensor_scalar_add(xin[k][:], u[:], bap("be2", k))
                else:
                    def extra(k, u, eng, xin=xin):
                        eng.tensor_scalar(xin[k][:], u[:], WS,
                                          bap("be2s", k), OP.mult, OP.add)
                layer_norm(xout, lambda k: bap("g2", k), x8w, extra)

            for k in range(KT):
                nc.sync.dma_start(yT_out[P * k:P * (k + 1), :], xin[k][:])

    bacc_mod.get_activation_tables = _patched
    hw_specs.get_activation_tables = _patched
    try:
        nc.compile()
    finally:
        bacc_mod.get_activation_tables = _orig_tables
        hw_specs.get_activation_tables = _orig_tables
    return nc


def _host_prep(inputs, nb):
    import ml_dtypes
    f8t = ml_dtypes.float8_e4m3fn
    qkv_w = np.asarray(inputs["qkv_w"], dtype=np.float32)[:nb]
    qkv_b = np.asarray(inputs["qkv_b"], dtype=np.float32)[:nb]
    fc_w = np.asarray(inputs["fc_w"], dtype=np.float32)[:nb]
    fc_b = np.asarray(inputs["fc_b"], dtype=np.float32)[:nb]
    w1 = np.asarray(inputs["w1"], dtype=np.float32)[:nb]
    b1 = np.asarray(inputs["b1"], dtype=np.float32)[:nb]
    w2 = np.asarray(inputs["w2"], dtype=np.float32)[:nb]
    b2 = np.asarray(inputs["b2"], dtype=np.float32)[:nb]
    g1 = np.asarray(inputs["ln1_g"], dtype=np.float32)[:nb]
    be1 = np.asarray(inputs["ln1_b"], dtype=np.float32)[:nb]
    g2 = np.asarray(inputs["ln2_g"], dtype=np.float32)[:nb]
    be2 = np.asarray(inputs["ln2_b"], dtype=np.float32)[:nb]

    idx_q = np.concatenate([np.arange(192 * h, 192 * h + 64)
                            for h in range(H)])
    idx_k = idx_q + 64
    idx_v = idx_q + 128

    def btile(b, nt):  # [nb, N] -> [nb, P, nt] with [l, p, m] = b[l, 128m+p]
        return b.reshape(nb, nt, P).transpose(0, 2, 1)

    wq_s = qkv_w[:, :, idx_q]
    wk_s = qkv_w[:, :, idx_k]
    wv_s = qkv_w[:, :, idx_v]
    # LN output biases are not applied on-device; fold them into the next
    # layer's biases. Block l>0 inputs x8 lack be2[l-1]; LN1 output lacks be1.
    dt64 = np.float64
    prev_be2 = np.concatenate(
        [np.zeros((1, D), np.float32), be2[:-1]], axis=0).astype(dt64)
    bq_eff = qkv_b[:, idx_q] + np.einsum(
        "ld,ldf->lf", prev_be2, wq_s.astype(dt64)).astype(np.float32)
    bk_eff = qkv_b[:, idx_k] + np.einsum(
        "ld,ldf->lf", prev_be2, wk_s.astype(dt64)).astype(np.float32)
    bv_eff = qkv_b[:, idx_v] + np.einsum(
        "ld,ldf->lf", prev_be2, wv_s.astype(dt64)).astype(np.float32)
    fcb_eff = fc_b + np.einsum("ld,ldf->lf", bv_eff.astype(dt64),
                               fc_w.astype(dt64)).astype(np.float32)
    b1_eff = b1 + np.einsum("ld,ldf->lf", be1.astype(dt64),
                            w1.astype(dt64)).astype(np.float32)
    biases = np.concatenate([
        btile(bq_eff * WS, 4), btile(bk_eff * WS, 4),
        btile(fcb_eff * WS, 4), btile(b2 * WS, 4), btile(g1, 4),
        btile(be1, 4), btile(g2, 4), btile(be2, 4), btile(b1_eff * WS, 8),
        btile(be2 * WS, 4)], axis=2)

    def w8(a):
        return np.ascontiguousarray(a * WS).astype(f8t)

    common = {
        "wq": w8(wq_s),
        "wk": w8(wk_s),
        "wv": w8(wv_s),
        "fcw": w8(fc_w),
        "w1": np.ascontiguousarray(w1 * WS).astype(np.float16),
        "w2": np.ascontiguousarray(w2 * WS).astype(np.float16),
        "biases": np.ascontiguousarray(biases),
    }
    X = np.asarray(inputs["X"], dtype=np.float32)
    in_maps = []
    for c in range(N_CORES):
        b, r = c // 4, c % 4
        xT = np.ascontiguousarray(X[b, TOK * r:TOK * (r + 1), :].T) * WS
        in_maps.append({"xT": xT, **common})
    return in_maps


def get_nc(nb=NB, reps=1):
    key = (nb, reps)
    if key not in _CACHE:
        _CACHE[key] = _build(nb, reps)
    return _CACHE[key]


def kernel(**inputs):
    from concourse.bass_utils import run_bass_kernel_spmd

    nb = NB
    nc = get_nc(nb)
    in_maps = _host_prep(inputs, nb)
    res = run_bass_kernel_spmd(nc, in_maps, list(range(N_CORES)))
    Y = np.zeros((B, S, D), dtype=np.float32)
    for c in range(N_CORES):
        b, r = c // 4, c % 4
        Y[b, TOK * r:TOK * (r + 1), :] = res.results[c]["yT"].T
    return Y


# revision 7
# speedup vs baseline: 1.0082x; 1.0082x over previous
"""12-block transformer encoder (B=2, S=2048, D=512, H=8, DHID=1024) on 8 trn2 cores.

Sequence-parallel: core c owns batch c//4, tokens 512*(c%4)..+512. Weights
replicated, fp8e4m3 at 64x scale (escapes e4m3 denormals; folds undone in
existing scalar slots). Residual stream f32 at 64x. Projections + attn@V run
fp8 DoubleRow (2 contraction chunks per matmul, 0.5 cyc/row). Scores fp8
1 cyc/row. Softmax exp split across engines: exact Exp on Activation, bit-trick
fast-exp (tensor_scalar -> uint8 bits viewed as fp8e4) on Pool and DVE,
interleaved per score-group so all three engines run concurrently.
LayerNorm rstd = Exp(-0.5*Ln(D*var)+lnC) on Activation; a get_activation_tables
patch during build makes the table pass pick the single set containing
exp+ln+relu, so there is exactly one table load. LN output biases are folded
into the next layer's biases host-side. K^T and aug-V AllGathered in fp8 as
two collectives (K first so scores start sooner; ones columns ride with V so
the softmax denominator falls out of the attn@V matmul).
"""
import sys
import numpy as np

for _p in ("/opt/trn_rl_repo", "/root/.axon_site/_ro/trn_rl_repo"):
    if _p not in sys.path:
        sys.path.insert(0, _p)

P = 128
B, S, D = 2, 2048, 512
H, DH, DHID = 8, 64, 1024
NB = 12
TOK = 512            # tokens per core
KT = D // P          # 4 contraction chunks over D
N_CORES = 8
GROUPS = [[0, 1, 2, 3], [4, 5, 6, 7]]
VW = 768             # aug-v row width per token ptile (4 pairs x 192)
WS = 64.0            # weight scale
LOG2E = 1.4426950408889634
SPLIT_AG = False

_CACHE = {}


def _build(nb, reps=1):
    import os
    import concourse.bass as bass
    import concourse.mybir as mybir
    import concourse.tile as tile
    from concourse import bacc
    import concourse.hw_specs as hw_specs
    import concourse.bacc as bacc_mod

    f32 = mybir.dt.float32
    f16 = mybir.dt.float16
    f8 = mybir.dt.float8e4
    u8 = mybir.dt.uint8
    AF = mybir.ActivationFunctionType
    OP = mybir.AluOpType
    DR = mybir.MatmulPerfMode.DoubleRow

    # Steer the act-table pass to the one set holding exp+ln+relu so the
    # kernel needs a single table load. Runtime-correct: we only narrow the
    # claimed contents of the other sets; ids stay the json indices.
    _orig_tables = hw_specs.get_activation_tables

    def _patched(arch):
        t = _orig_tables(arch)
        for name, s in t.items():
            if name != "natural_log_exp_and_others":
                s.discard(AF.Exp)
                s.discard(AF.Ln)
                s.discard(AF.Relu)
        return t

    sim1 = os.environ.get("KSIM") in ("1", "2")
    sim_cheap = os.environ.get("KSIM") == "2"
    nc = bacc.Bacc("TRN2", target_bir_lowering=False, debug=False,
                   num_devices=(1 if sim1 else N_CORES))

    xT_in = nc.declare_dram_parameter("xT", [D, TOK], f32, isOutput=False)
    wq_in = nc.declare_dram_parameter("wq", [nb, D, D], f8, isOutput=False)
    wk_in = nc.declare_dram_parameter("wk", [nb, D, D], f8, isOutput=False)
    wv_in = nc.declare_dram_parameter("wv", [nb, D, D], f8, isOutput=False)
    fc_in = nc.declare_dram_parameter("fcw", [nb, D, D], f8, isOutput=False)
    w1_in = nc.declare_dram_parameter("w1", [nb, D, DHID], f16, isOutput=False)
    w2_in = nc.declare_dram_parameter("w2", [nb, DHID, D], f16, isOutput=False)
    bias_in = nc.declare_dram_parameter("biases", [nb, P, 44], f32,
                                        isOutput=False)
    yT_out = nc.declare_dram_parameter("yT", [D, TOK], f32, isOutput=True)

    inv_sqrt_d = float(1.0 / np.sqrt(D))
    SEXP = inv_sqrt_d / (WS * WS)     # exp arg = psum * SEXP
    FE_A = LOG2E * 8.0 * SEXP         # fast-exp bits = psum*FE_A + FE_B
    FE_B = 7.0 * 8.0 - 0.58
    BOF = {"bq": 0, "bk": 4, "fcb": 8, "b2": 12, "g1": 16, "be1": 20,
           "g2": 24, "be2": 28, "b1": 32, "be2s": 40}

    with tile.TileContext(nc) as tc:
        with tc.tile_pool(name="w", bufs=2) as sbw, \
             tc.tile_pool(name="bias", bufs=2) as sbb, \
             tc.tile_pool(name="act", bufs=1) as sba, \
             tc.tile_pool(name="e", bufs=8) as sbe, \
             tc.tile_pool(name="pers", bufs=1) as sbp, \
             tc.tile_pool(name="psS", bufs=2, space="PSUM") as psS, \
             tc.tile_pool(name="psA", bufs=4, space="PSUM") as psA, \
             tc.tile_pool(name="dram", bufs=2, space="DRAM") as dram:

            ones16 = sbp.tile([P, P], f16, tag="ones16", name="ones16")
            nc.vector.memset(ones16[:], 1.0)
            lnC = sbp.tile([P, 1], f32, tag="lnC", name="lnC")
            nc.vector.memset(lnC[:], float(np.log(np.sqrt(D) / WS)))
            ones8 = sbp.tile([P, 2 * P], f8, tag="ones8", name="ones8")
            nc.vector.memset(ones8[:], 1.0)
            on2 = ones8[:].rearrange("p (t m) -> p t m", t=2)
            ktf = [sbp.tile([P, S], f8, tag=f"ktf{p}", name=f"ktf{p}")
                   for p in range(4)]
            # all 16 key-ptile aug-v tiles in one tensor: [P, 16*768] fp8
            va_all = sbp.tile([P, 16 * VW], f8, tag="va", name="va")
            # own-token aug-v staging: [P, 4*768], ones written once
            v_own = sbp.tile([P, 4 * VW], f8, tag="vown", name="vown")
            for tt_ in (va_all, v_own):
                oc = tt_[:].rearrange("p (a b) -> p a b", b=64)
                nc.vector.memset(oc[:, 1:oc.shape[1]:3, :], 1.0)

            # residual stream (64x) + fp8 copy
            xin = [sba.tile([P, TOK], f32, tag=f"x32_{k}", name=f"x32_{k}")
                   for k in range(KT)]
            x8w = sba.tile([P, KT * TOK], f8, tag="x8w", name="x8w")
            for k in range(KT):
                nc.sync.dma_start(xin[k][:], xT_in[P * k:P * (k + 1), :])
                nc.vector.tensor_scalar_mul(
                    x8w[:, TOK * k:TOK * (k + 1)], xin[k][:], 1.0 / WS)

            def pair2(ap_wide, k, width, lo, hi):
                """[P, 2, hi-lo] view: chunks k,k+1 of ap_wide, cols lo:hi."""
                r = ap_wide.rearrange("p (k c) -> p k c", c=width)
                return r[:, k:k + 2, lo:hi]

            def layer_norm(x4, g_ap, ln8_out, out_extra):
                """x4: 4 f32 [P,TOK] tiles at 64x. Writes normalized*g (no
                +bias: folded downstream) as fp8 into ln8_out (wide); calls
                out_extra(k, u32, eng) for extra outputs (u32 f32)."""
                m8 = sba.tile([P, KT * TOK], f8, tag="ln_m8", name="ln_m8")
                sq8 = sba.tile([P, KT * TOK], f8, tag="ln_sq8", name="ln_sq8")
                for k in range(KT):
                    cs = slice(TOK * k, TOK * (k + 1))
                    eng = (nc.gpsimd, nc.vector)[k % 2]
                    eng.tensor_scalar_mul(m8[:, cs], x4[k][:], 1.0 / WS)
                    eng.tensor_mul(sq8[:, cs], m8[:, cs], m8[:, cs])
                psum_s = psA.tile([P, TOK], f32, tag="ps", name="ps")
                psum_q = psA.tile([P, TOK], f32, tag="ps", name="ps")
                for k in (0, 2):
                    nc.tensor.matmul(psum_s[:], on2,
                                     pair2(m8[:], k, TOK, 0, TOK),
                                     start=(k == 0), stop=(k == 2),
                                     perf_mode=DR)
                    nc.tensor.matmul(psum_q[:], on2,
                                     pair2(sq8[:], k, TOK, 0, TOK),
                                     start=(k == 0), stop=(k == 2),
                                     perf_mode=DR)
                a = sba.tile([P, TOK], f32, tag="ln_a", name="ln_a")
                nc.vector.tensor_scalar_mul(a[:], psum_s[:], WS / D)
                a2 = sba.tile([P, TOK], f32, tag="ln_a2", name="ln_a2")
                nc.gpsimd.tensor_mul(a2[:], a[:], a[:])
                pt = sba.tile([P, TOK], f32, tag="ln_pt", name="ln_pt")
                nc.vector.scalar_tensor_tensor(
                    pt[:], in0=a2[:], scalar=float(-D / (WS * WS)),
                    in1=psum_q[:], op0=OP.mult, op1=OP.add)
                # rstd*C = exp(-0.5*ln(pt) + ln(C)), C = sqrt(D)/WS
                lnv = sba.tile([P, TOK], f32, tag="ln_lnv", name="ln_lnv")
                nc.scalar.activation(lnv[:], pt[:], AF.Ln, bias=0.0, scale=1.0)
                rstd = sba.tile([P, TOK], f32, tag="ln_rstd", name="ln_rstd")
                nc.scalar.activation(rstd[:], lnv[:], AF.Exp,
                                     bias=lnC[:, 0:1], scale=-0.5)
                for k in range(KT):
                    ei = k % 2
                    eng = (nc.vector, nc.gpsimd)[ei]
                    t = sba.tile([P, TOK], f32, tag=f"ln_t{ei}",
                                 name=f"ln_t{ei}")
                    eng.tensor_sub(t[:], x4[k][:], a[:])

                    def stt_mm(out_ap, tin):
                        # (t * g) * rstd; Pool lacks scalar_tensor_tensor
                        if ei == 0:
                            eng.scalar_tensor_tensor(
                                out_ap, in0=tin, scalar=g_ap(k), in1=rstd[:],
                                op0=OP.mult, op1=OP.mult)
                        else:
                            w = sba.tile([P, TOK], f32, tag="ln_w1",
                                         name="ln_w1")
                            eng.tensor_scalar_mul(w[:], tin, g_ap(k))
                            eng.tensor_mul(out_ap, w[:], rstd[:])

                    if out_extra is None:
                        stt_mm(ln8_out[:, TOK * k:TOK * (k + 1)], t[:])
                    else:
                        u = sba.tile([P, TOK], f32, tag=f"ln_u{ei}",
                                     name=f"ln_u{ei}")
                        stt_mm(u[:], t[:])
                        eng.tensor_copy(ln8_out[:, TOK * k:TOK * (k + 1)],
                                        u[:])
                        out_extra(k, u, eng)

            for rep in range(reps):
              for l in range(nb):
                # ---- weights: one DMA per tensor (fp8) ----
                wq_t = sbw.tile([P, KT * D], f8, tag="wq", name="wq")
                wk_t = sbw.tile([P, KT * D], f8, tag="wk", name="wk")
                wv_t = sbw.tile([P, KT * D], f8, tag="wv", name="wv")
                fc_t = sbw.tile([P, KT * D], f8, tag="fcw", name="fcw")
                w1_t = sbw.tile([P, KT * DHID], f16, tag="w1", name="w1")
                w2_t = sbw.tile([P, 8 * D], f16, tag="w2", name="w2")
                for sb_t, src, width in ((wk_t, wk_in, D), (wv_t, wv_in, D),
                                         (wq_t, wq_in, D), (fc_t, fc_in, D),
                                         (w1_t, w1_in, DHID)):
                    nc.sync.dma_start(
                        sb_t[:].rearrange("p (k c) -> p k c", c=width),
                        src[l].rearrange("(k p) c -> p k c", p=P))
                nc.sync.dma_start(
                    w2_t[:].rearrange("p (k c) -> p k c", c=D),
                    w2_in[l].rearrange("(k p) c -> p k c", p=P))
                bt = sbb.tile([P, 44], f32, tag="bias", name="bias")
                nc.sync.dma_start(bt[:], bias_in[l, :, :])

                def bap(name, idx):
                    o = BOF[name] + idx
                    return bt[:, o:o + 1]

                # ---- k^T (feature-major, fp8 at 64x), AG_K ----
                kT8 = sba.tile([P, KT * TOK], f8, tag="kT8", name="kT8")
                for hp in range(4):
                    ps = psA.tile([P, TOK], f32, tag="ps", name="ps")
                    for k in (0, 2):
                        nc.tensor.matmul(
                            ps[:], pair2(wk_t[:], k, D, P * hp, P * (hp + 1)),
                            pair2(x8w[:], k, TOK, 0, TOK),
                            start=(k == 0), stop=(k == 2), perf_mode=DR)
                    nc.scalar.activation(
                        kT8[:, TOK * hp:TOK * (hp + 1)], ps[:], AF.Identity,
                        bias=bap("bk", hp), scale=1.0)
                cc_i = dram.tile([D, TOK + VW], f8, tag="cc_i", name="cc_i")
                cc_o = dram.tile([4 * D, TOK + VW], f8, tag="cc_o",
                                 name="cc_o")
                cc_ok = cc_o[:, 0:TOK]
                cc_ov = cc_o[:, TOK:TOK + VW]
                nc.sync.dma_start(
                    cc_i[:, 0:TOK].rearrange("(hp p) c -> p hp c", p=P),
                    kT8[:].rearrange("p (hp c) -> p hp c", c=TOK))

                # ---- v (token-major, aug fp8 at 64x), AG_V ----
                for t in range(4):
                    ps = psA.tile([P, D], f32, tag="ps", name="ps")
                    for k in (0, 2):
                        nc.tensor.matmul(
                            ps[:], pair2(x8w[:], k, TOK, P * t, P * (t + 1)),
                            pair2(wv_t[:], k, D, 0, D),
                            start=(k == 0), stop=(k == 2), perf_mode=DR)
                    # one 4D-AP copy: heads (2p, 2p+1) -> aug slots (0, 2)
                    ps4 = ps[:].rearrange("p (pr w b) -> p pr w b", pr=4, w=2)
                    vo4 = v_own[:, VW * t:VW * (t + 1)].rearrange(
                        "p (pr w2 b) -> p pr w2 b", pr=4, w2=3)[:, :, 0:3:2, :]
                    nc.scalar.activation(vo4, ps4[:, :, :, :], AF.Copy,
                                         bias=0.0, scale=1.0)
                nc.sync.dma_start(
                    cc_i[:, TOK:TOK + VW].rearrange("(t p) c -> p t c", p=P),
                    v_own[:].rearrange("p (t c) -> p t c", c=VW))
                if sim1:
                    for r in range(4):
                        nc.sync.dma_start(cc_o[D * r:D * (r + 1), :],
                                          cc_i[:, :])
                else:
                    nc.gpsimd.collective_compute(
                        "AllGather", mybir.AluOpType.bypass,
                        replica_groups=GROUPS,
                        ins=[cc_i[:].opt()], outs=[cc_o[:].opt()])

                # ---- q^T fp8 (overlaps the collectives) ----
                qT8 = sba.tile([P, KT * TOK], f8, tag="qT8", name="qT8")
                for hp in range(4):
                    ps = psA.tile([P, TOK], f32, tag="ps", name="ps")
                    for k in (0, 2):
                        nc.tensor.matmul(
                            ps[:], pair2(wq_t[:], k, D, P * hp, P * (hp + 1)),
                            pair2(x8w[:], k, TOK, 0, TOK),
                            start=(k == 0), stop=(k == 2), perf_mode=DR)
                    nc.vector.tensor_scalar_add(
                        qT8[:, TOK * hp:TOK * (hp + 1)], ps[:], bap("bq", hp))

                # ---- gather-in loads ----
                for p in range(4):
                    nc.sync.dma_start(
                        ktf[p][:].rearrange("p (c w) -> p c w", w=TOK),
                        cc_ok.rearrange(
                            "(c p q) w -> c p q w", p=4, q=P)[:, p, :, :]
                        .rearrange("c q w -> q c w"))
                for cch in range(4):
                    nc.sync.dma_start(
                        va_all[:, VW * 4 * cch:VW * 4 * (cch + 1)]
                        .rearrange("p (j c) -> p j c", c=VW),
                        cc_ov[D * cch:D * (cch + 1), :]
                        .rearrange("(j p) c -> p j c", p=P))

                # ---- attention, head pairs ----
                oT8w = sba.tile([P, KT * TOK], f8, tag="oT8w", name="oT8w")
                va_r = va_all[:].rearrange("p (j c) -> p j c", c=VW)
                for hp in range(4):
                    qs = qT8[:, TOK * hp:TOK * (hp + 1)]
                    po_e = psA.tile([P, TOK], f32, tag="ps", name="ps")
                    po_o = psA.tile([P, TOK], f32, tag="ps", name="ps")
                    e_list = []
                    for g in range(8):
                        ps_e = psS.tile([P, 1024], f32, tag="ps_sc",
                                        name="ps_sc")
                        ps_o = psS.tile([P, 1024], f32, tag="ps_sc",
                                        name="ps_sc")
                        for c in range(2):
                            j = 2 * g + c
                            nc.tensor.matmul(
                                ps_e[:, TOK * c:TOK * (c + 1)],
                                ktf[hp][0:64, P * j:P * (j + 1)],
                                qs[0:64, :], start=True, stop=True)
                            nc.tensor.matmul(
                                ps_o[:, TOK * c:TOK * (c + 1)],
                                ktf[hp][64:128, P * j:P * (j + 1)],
                                qs[64:128, :], start=True, stop=True)
                        e_e = sbe.tile([P, 1024], u8, tag="e", name="e")
                        e_o = sbe.tile([P, 1024], u8, tag="e", name="e")
                        # interleave: Act every g; Pool/DVE alternate
                        nc.scalar.activation(e_e[:].bitcast(f8), ps_e[:],
                                             AF.Exp, bias=0.0, scale=SEXP)
                        if g == 0:
                            nc.scalar.activation(e_o[:].bitcast(f8), ps_o[:],
                                                 AF.Exp, bias=0.0, scale=SEXP)
                        else:
                            nc.vector.tensor_scalar(e_o[:], ps_o[:], FE_A,
                                                    FE_B, OP.mult, OP.add)
                        e_list.append((e_e, e_o))
                        if g >= 1:
                            pe_, po_ = e_list[g - 1]
                            jg = g - 1
                            nc.tensor.matmul(
                                po_e[:],
                                va_r[:, 2 * jg:2 * jg + 2,
                                     192 * hp:192 * hp + 128],
                                pe_[:].bitcast(f8).rearrange(
                                    "p (t c) -> p t c", t=2),
                                start=(jg == 0), stop=False, perf_mode=DR)
                            nc.tensor.matmul(
                                po_o[:],
                                va_r[:, 2 * jg:2 * jg + 2,
                                     192 * hp + 64:192 * hp + 192],
                                po_[:].bitcast(f8).rearrange(
                                    "p (t c) -> p t c", t=2),
                                start=(jg == 0), stop=False, perf_mode=DR)
                    pe_, po_ = e_list[7]
                    nc.tensor.matmul(
                        po_e[:], va_r[:, 14:16, 192 * hp:192 * hp + 128],
                        pe_[:].bitcast(f8).rearrange("p (t c) -> p t c", t=2),
                        start=False, stop=True, perf_mode=DR)
                    nc.tensor.matmul(
                        po_o[:], va_r[:, 14:16, 192 * hp + 64:192 * hp + 192],
                        po_[:].bitcast(f8).rearrange("p (t c) -> p t c", t=2),
                        start=False, stop=True, perf_mode=DR)
                    # normalize: denominators are rows dl; o rows ol
                    for par, po in ((0, po_e), (1, po_o)):
                        ol = slice(64 * par, 64 * par + 64)
                        dl = slice(64 * (1 - par), 64 * (1 - par) + 64)
                        dcp = sba.tile([P, TOK], f16, tag=f"dcp{par}",
                                       name=f"dcp{par}")
                        nc.scalar.activation(dcp[dl, :], po[dl, :], AF.Copy,
                                             bias=0.0, scale=1.0 / 64.0)
                        ps2 = psA.tile([P, TOK], f32, tag="ps", name="ps")
                        nc.tensor.matmul(ps2[:], ones16[dl, :], dcp[dl, :],
                                         start=True, stop=True)
                        rec = sba.tile([P, TOK], f32, tag=f"rec{par}",
                                       name=f"rec{par}")
                        nc.vector.reciprocal_approx_fast(rec[:], ps2[:])
                        nc.vector.scalar_tensor_tensor(
                            oT8w[:, TOK * hp:TOK * (hp + 1)][ol, :],
                            in0=po[ol, :], scalar=float(1.0 / WS),
                            in1=rec[ol, :], op0=OP.mult, op1=OP.mult)

                # ---- fc + residual (xmid = 64x) ----
                xmid = []
                for m in range(4):
                    ps = psA.tile([P, TOK], f32, tag="ps", name="ps")
                    for k in (0, 2):
                        nc.tensor.matmul(
                            ps[:], pair2(fc_t[:], k, D, P * m, P * (m + 1)),
                            pair2(oT8w[:], k, TOK, 0, TOK),
                            start=(k == 0), stop=(k == 2), perf_mode=DR)
                    xm = sba.tile([P, TOK], f32, tag=f"xmid{m}",
                                  name=f"xmid{m}")
                    nc.vector.scalar_tensor_tensor(
                        xm[:], in0=ps[:], scalar=bap("fcb", m), in1=xin[m][:],
                        op0=OP.add, op1=OP.add)
                    xmid.append(xm)

                # ---- LN1 -> MLP ----
                ln8w = sba.tile([P, KT * TOK], f16, tag="ln8w", name="ln8w")
                layer_norm(xmid, lambda k: bap("g1", k), ln8w, None)
                h8w = sba.tile([P, 8 * TOK], f16, tag="h8w", name="h8w")
                for m in range(8):
                    ps = psA.tile([P, TOK], f32, tag="ps", name="ps")
                    for k in range(KT):
                        nc.tensor.matmul(
                            ps[:],
                            w1_t[:, DHID * k + P * m:DHID * k + P * (m + 1)],
                            ln8w[:, TOK * k:TOK * (k + 1)],
                            start=(k == 0), stop=(k == KT - 1))
                    nc.scalar.activation(h8w[:, TOK * m:TOK * (m + 1)], ps[:],
                                         AF.Relu, bias=bap("b1", m), scale=1.0)
                xout = []
                for m in range(4):
                    ps = psA.tile([P, TOK], f32, tag="ps", name="ps")
                    for k in range(8):
                        nc.tensor.matmul(
                            ps[:], w2_t[:, D * k + P * m:D * k + P * (m + 1)],
                            h8w[:, TOK * k:TOK * (k + 1)],
                            start=(k == 0), stop=(k == 7))
                    tb = sba.tile([P, TOK], f32, tag=f"tb{m}", name=f"tb{m}")
                    nc.vector.tensor_scalar(tb[:], ps[:], 1.0 / WS,
                                            bap("b2", m), OP.mult, OP.add)
                    xo = sba.tile([P, TOK], f32, tag=f"xout{m}",
                                  name=f"xout{m}")
                    nc.gpsimd.tensor_add(xo[:], tb[:], xmid[m][:])
                    xout.append(xo)

                # ---- LN2 -> next block's x (fp8 sans bias + f32@64x) ----
                x8w = sba.tile([P, KT * TOK], f8, tag="x8w", name="x8w")
                xin = [sba.tile([P, TOK], f32, tag=f"x32_{k}",
                                name=f"x32_{k}") for k in range(KT)]
                last = (rep == reps - 1) and (l == nb - 1)

                if last:
                    def extra(k, u, eng, xin=xin):
                        eng.tensor_scalar_add(xin[k][:], u[:], bap("be2", k))
                else:
                    def extra(k, u, eng, xin=xin):
                        eng.tensor_scalar(xin[k][:], u[:], WS,
                                          bap("be2s", k), OP.mult, OP.add)
                layer_norm(xout, lambda k: bap("g2", k), x8w, extra)

            for k in range(KT):
                nc.sync.dma_start(yT_out[P * k:P * (k + 1), :], xin[k][:])

    bacc_mod.get_activation_tables = _patched
    hw_specs.get_activation_tables = _patched
    try:
        nc.compile()
    finally:
        bacc_mod.get_activation_tables = _orig_tables
        hw_specs.get_activation_tables = _orig_tables
    return nc


def _host_prep(inputs, nb):
    import ml_dtypes
    f8t = ml_dtypes.float8_e4m3fn
    qkv_w = np.asarray(inputs["qkv_w"], dtype=np.float32)[:nb]
    qkv_b = np.asarray(inputs["qkv_b"], dtype=np.float32)[:nb]
    fc_w = np.asarray(inputs["fc_w"], dtype=np.float32)[:nb]
    fc_b = np.asarray(inputs["fc_b"], dtype=np.float32)[:nb]
    w1 = np.asarray(inputs["w1"], dtype=np.float32)[:nb]
    b1 = np.asarray(inputs["b1"], dtype=np.float32)[:nb]
    w2 = np.asarray(inputs["w2"], dtype=np.float32)[:nb]
    b2 = np.asarray(inputs["b2"], dtype=np.float32)[:nb]
    g1 = np.asarray(inputs["ln1_g"], dtype=np.float32)[:nb]
    be1 = np.asarray(inputs["ln1_b"], dtype=np.float32)[:nb]
    g2 = np.asarray(inputs["ln2_g"], dtype=np.float32)[:nb]
    be2 = np.asarray(inputs["ln2_b"], dtype=np.float32)[:nb]

    idx_q = np.concatenate([np.arange(192 * h, 192 * h + 64)
                            for h in range(H)])
    idx_k = idx_q + 64
    idx_v = idx_q + 128

    def btile(b, nt):  # [nb, N] -> [nb, P, nt] with [l, p, m] = b[l, 128m+p]
        return b.reshape(nb, nt, P).transpose(0, 2, 1)

    wq_s = qkv_w[:, :, idx_q]
    wk_s = qkv_w[:, :, idx_k]
    wv_s = qkv_w[:, :, idx_v]
    # LN output biases are not applied on-device; fold them into the next
    # layer's biases. Block l>0 inputs x8 lack be2[l-1]; LN1 output lacks be1.
    dt64 = np.float64
    prev_be2 = np.concatenate(
        [np.zeros((1, D), np.float32), be2[:-1]], axis=0).astype(dt64)
    bq_eff = qkv_b[:, idx_q] + np.einsum(
        "ld,ldf->lf", prev_be2, wq_s.astype(dt64)).astype(np.float32)
    bk_eff = qkv_b[:, idx_k] + np.einsum(
        "ld,ldf->lf", prev_be2, wk_s.astype(dt64)).astype(np.float32)
    bv_eff = qkv_b[:, idx_v] + np.einsum(
        "ld,ldf->lf", prev_be2, wv_s.astype(dt64)).astype(np.float32)
    fcb_eff = fc_b + np.einsum("ld,ldf->lf", bv_eff.astype(dt64),
                               fc_w.astype(dt64)).astype(np.float32)
    b1_eff = b1 + np.einsum("ld,ldf->lf", be1.astype(dt64),
                            w1.astype(dt64)).astype(np.float32)
    biases = np.concatenate([
        btile(bq_eff * WS, 4), btile(bk_eff * WS, 4),
        btile(fcb_eff * WS, 4), btile(b2 * WS, 4), btile(g1, 4),
        btile(be1, 4), btile(g2, 4), btile(be2, 4), btile(b1_eff * WS, 8),
        btile(be2 * WS, 4)], axis=2)

    def w8(a):
        return np.ascontiguousarray(a * WS).astype(f8t)

    common = {
        "wq": w8(wq_s),
        "wk": w8(wk_s),
        "wv": w8(wv_s),
        "fcw": w8(fc_w),
        "w1": np.ascontiguousarray(w1 * WS).astype(np.float16),
        "w2": np.ascontiguousarray(w2 * WS).astype(np.float16),
        "biases": np.ascontiguousarray(biases),
    }
    X = np.asarray(inputs["X"], dtype=np.float32)
    in_maps = []
    for c in range(N_CORES):
        b, r = c // 4, c % 4
        xT = np.ascontiguousarray(X[b, TOK * r:TOK * (r + 1), :].T) * WS
        in_maps.append({"xT": xT, **common})
    return in_maps


def get_nc(nb=NB, reps=1):
    key = (nb, reps)
    if key not in _CACHE:
        _CACHE[key] = _build(nb, reps)
    return _CACHE[key]


def kernel(**inputs):
    from concourse.bass_utils import run_bass_kernel_spmd

    nb = NB
    nc = get_nc(nb)
    in_maps = _host_prep(inputs, nb)
    res = run_bass_kernel_spmd(nc, in_maps, list(range(N_CORES)))
    Y = np.zeros((B, S, D), dtype=np.float32)
    for c in range(N_CORES):
        b, r = c // 4, c % 4
        Y[b, TOK * r:TOK * (r + 1), :] = res.results[c]["yT"].T
    return Y


# revision 9
# speedup vs baseline: 1.0238x; 1.0155x over previous
"""12-block transformer encoder (B=2, S=2048, D=512, H=8, DHID=1024) on 8 trn2 cores.

Sequence-parallel: core c owns batch c//4, tokens 512*(c%4)..+512. Weights
replicated, fp8e4m3 at 64x scale (escapes e4m3 denormals; folds undone in
existing scalar slots). Residual stream f32 at 64x. Projections + attn@V run
fp8 DoubleRow (2 contraction chunks per matmul, 0.5 cyc/row). Scores fp8
1 cyc/row. Softmax exp split across engines: exact Exp on Activation, bit-trick
fast-exp (tensor_scalar -> uint8 bits viewed as fp8e4) on Pool and DVE,
interleaved per score-group so all three engines run concurrently.
LayerNorm rstd = Exp(-0.5*Ln(D*var)+lnC) on Activation; a get_activation_tables
patch during build makes the table pass pick the single set containing
exp+ln+relu, so there is exactly one table load. LN output biases are folded
into the next layer's biases host-side. K^T and aug-V AllGathered in fp8 as
two collectives (K first so scores start sooner; ones columns ride with V so
the softmax denominator falls out of the attn@V matmul).
"""
import sys
import numpy as np

for _p in ("/opt/trn_rl_repo", "/root/.axon_site/_ro/trn_rl_repo"):
    if _p not in sys.path:
        sys.path.insert(0, _p)

P = 128
B, S, D = 2, 2048, 512
H, DH, DHID = 8, 64, 1024
NB = 12
TOK = 512            # tokens per core
KT = D // P          # 4 contraction chunks over D
N_CORES = 8
GROUPS = [[0, 1, 2, 3], [4, 5, 6, 7]]
VW = 768             # aug-v row width per token ptile (4 pairs x 192)
WS = 64.0            # weight scale
LOG2E = 1.4426950408889634
SPLIT_AG = False

_CACHE = {}


def _build(nb, reps=1):
    import os
    import concourse.bass as bass
    import concourse.mybir as mybir
    import concourse.tile as tile
    from concourse import bacc
    import concourse.hw_specs as hw_specs
    import concourse.bacc as bacc_mod

    f32 = mybir.dt.float32
    f16 = mybir.dt.float16
    f8 = mybir.dt.float8e4
    u8 = mybir.dt.uint8
    AF = mybir.ActivationFunctionType
    OP = mybir.AluOpType
    DR = mybir.MatmulPerfMode.DoubleRow

    # Steer the act-table pass to the one set holding exp+ln+relu so the
    # kernel needs a single table load. Runtime-correct: we only narrow the
    # claimed contents of the other sets; ids stay the json indices.
    _orig_tables = hw_specs.get_activation_tables

    def _patched(arch):
        t = _orig_tables(arch)
        for name, s in t.items():
            if name != "natural_log_exp_and_others":
                s.discard(AF.Exp)
                s.discard(AF.Ln)
                s.discard(AF.Relu)
        return t

    sim1 = os.environ.get("KSIM") in ("1", "2")
    sim_cheap = os.environ.get("KSIM") == "2"
    nc = bacc.Bacc("TRN2", target_bir_lowering=False, debug=False,
                   num_devices=(1 if sim1 else N_CORES))

    xT_in = nc.declare_dram_parameter("xT", [D, TOK], f32, isOutput=False)
    wq_in = nc.declare_dram_parameter("wq", [nb, D, D], f8, isOutput=False)
    wk_in = nc.declare_dram_parameter("wk", [nb, D, D], f8, isOutput=False)
    wv_in = nc.declare_dram_parameter("wv", [nb, D, D], f8, isOutput=False)
    fc_in = nc.declare_dram_parameter("fcw", [nb, D, D], f8, isOutput=False)
    w1_in = nc.declare_dram_parameter("w1", [nb, D, DHID], f16, isOutput=False)
    w2_in = nc.declare_dram_parameter("w2", [nb, DHID, D], f16, isOutput=False)
    bias_in = nc.declare_dram_parameter("biases", [nb, P, 44], f32,
                                        isOutput=False)
    yT_out = nc.declare_dram_parameter("yT", [D, TOK], f32, isOutput=True)

    inv_sqrt_d = float(1.0 / np.sqrt(D))
    SEXP = inv_sqrt_d / (WS * WS)     # exp arg = psum * SEXP
    FE_A = LOG2E * 8.0 * SEXP         # fast-exp bits = psum*FE_A + FE_B
    FE_B = 7.0 * 8.0 - 0.58
    BOF = {"bq": 0, "bk": 4, "fcb": 8, "b2": 12, "g1": 16, "be1": 20,
           "g2": 24, "be2": 28, "b1": 32, "be2s": 40}

    with tile.TileContext(nc) as tc:
        with tc.tile_pool(name="w", bufs=2) as sbw, \
             tc.tile_pool(name="bias", bufs=2) as sbb, \
             tc.tile_pool(name="act", bufs=1) as sba, \
             tc.tile_pool(name="e", bufs=8) as sbe, \
             tc.tile_pool(name="pers", bufs=1) as sbp, \
             tc.tile_pool(name="psS", bufs=2, space="PSUM") as psS, \
             tc.tile_pool(name="psA", bufs=4, space="PSUM") as psA, \
             tc.tile_pool(name="dram", bufs=2, space="DRAM") as dram:

            ones16 = sbp.tile([P, P], f16, tag="ones16", name="ones16")
            nc.vector.memset(ones16[:], 1.0)
            lnC = sbp.tile([P, 1], f32, tag="lnC", name="lnC")
            nc.vector.memset(lnC[:], float(np.log(np.sqrt(D) / WS)))
            ones8 = sbp.tile([P, 2 * P], f8, tag="ones8", name="ones8")
            nc.vector.memset(ones8[:], 1.0)
            on2 = ones8[:].rearrange("p (t m) -> p t m", t=2)
            ktf = [sbp.tile([P, S], f8, tag=f"ktf{p}", name=f"ktf{p}")
                   for p in range(4)]
            # all 16 key-ptile aug-v tiles in one tensor: [P, 16*768] fp8
            va_all = sbp.tile([P, 16 * VW], f8, tag="va", name="va")
            # own-token aug-v staging: [P, 4*768], ones written once
            v_own = sbp.tile([P, 4 * VW], f8, tag="vown", name="vown")
            for tt_ in (va_all, v_own):
                oc = tt_[:].rearrange("p (a b) -> p a b", b=64)
                nc.vector.memset(oc[:, 1:oc.shape[1]:3, :], 1.0)

            # residual stream (64x) + fp8 copy
            xin = [sba.tile([P, TOK], f32, tag=f"x32_{k}", name=f"x32_{k}")
                   for k in range(KT)]
            x8w = sba.tile([P, KT * TOK], f8, tag="x8w", name="x8w")
            for k in range(KT):
                nc.sync.dma_start(xin[k][:], xT_in[P * k:P * (k + 1), :])
                nc.vector.tensor_scalar_mul(
                    x8w[:, TOK * k:TOK * (k + 1)], xin[k][:], 1.0 / WS)

            def pair2(ap_wide, k, width, lo, hi):
                """[P, 2, hi-lo] view: chunks k,k+1 of ap_wide, cols lo:hi."""
                r = ap_wide.rearrange("p (k c) -> p k c", c=width)
                return r[:, k:k + 2, lo:hi]

            def layer_norm(x4, g_ap, ln8_out, out_extra):
                """x4: 4 f32 [P,TOK] tiles at 64x. Writes normalized*g (no
                +bias: folded downstream) as fp8 into ln8_out (wide); calls
                out_extra(k, u32, eng) for extra outputs (u32 f32)."""
                m8 = sba.tile([P, KT * TOK], f8, tag="ln_m8", name="ln_m8")
                sq8 = sba.tile([P, KT * TOK], f8, tag="ln_sq8", name="ln_sq8")
                for k in range(KT):
                    cs = slice(TOK * k, TOK * (k + 1))
                    eng = (nc.gpsimd, nc.vector)[k % 2]
                    eng.tensor_scalar_mul(m8[:, cs], x4[k][:], 1.0 / WS)
                    eng.tensor_mul(sq8[:, cs], m8[:, cs], m8[:, cs])
                psum_s = psA.tile([P, TOK], f32, tag="ps", name="ps")
                psum_q = psA.tile([P, TOK], f32, tag="ps", name="ps")
                for k in (0, 2):
                    nc.tensor.matmul(psum_s[:], on2,
                                     pair2(m8[:], k, TOK, 0, TOK),
                                     start=(k == 0), stop=(k == 2),
                                     perf_mode=DR)
                    nc.tensor.matmul(psum_q[:], on2,
                                     pair2(sq8[:], k, TOK, 0, TOK),
                                     start=(k == 0), stop=(k == 2),
                                     perf_mode=DR)
                a = sba.tile([P, TOK], f32, tag="ln_a", name="ln_a")
                nc.vector.tensor_scalar_mul(a[:], psum_s[:], WS / D)
                a2 = sba.tile([P, TOK], f32, tag="ln_a2", name="ln_a2")
                nc.gpsimd.tensor_mul(a2[:], a[:], a[:])
                pt = sba.tile([P, TOK], f32, tag="ln_pt", name="ln_pt")
                nc.vector.scalar_tensor_tensor(
                    pt[:], in0=a2[:], scalar=float(-D / (WS * WS)),
                    in1=psum_q[:], op0=OP.mult, op1=OP.add)
                # rstd*C = exp(-0.5*ln(pt) + ln(C)), C = sqrt(D)/WS
                lnv = sba.tile([P, TOK], f32, tag="ln_lnv", name="ln_lnv")
                nc.scalar.activation(lnv[:], pt[:], AF.Ln, bias=0.0, scale=1.0)
                rstd = sba.tile([P, TOK], f32, tag="ln_rstd", name="ln_rstd")
                nc.scalar.activation(rstd[:], lnv[:], AF.Exp,
                                     bias=lnC[:, 0:1], scale=-0.5)
                for k in range(KT):
                    ei = k % 2
                    eng = (nc.vector, nc.gpsimd)[ei]
                    t = sba.tile([P, TOK], f32, tag=f"ln_t{ei}",
                                 name=f"ln_t{ei}")
                    eng.tensor_sub(t[:], x4[k][:], a[:])

                    def stt_mm(out_ap, tin):
                        # (t * g) * rstd; Pool lacks scalar_tensor_tensor
                        if ei == 0:
                            eng.scalar_tensor_tensor(
                                out_ap, in0=tin, scalar=g_ap(k), in1=rstd[:],
                                op0=OP.mult, op1=OP.mult)
                        else:
                            w = sba.tile([P, TOK], f32, tag="ln_w1",
                                         name="ln_w1")
                            eng.tensor_scalar_mul(w[:], tin, g_ap(k))
                            eng.tensor_mul(out_ap, w[:], rstd[:])

                    if out_extra is None:
                        stt_mm(ln8_out[:, TOK * k:TOK * (k + 1)], t[:])
                    else:
                        u = sba.tile([P, TOK], f32, tag=f"ln_u{ei}",
                                     name=f"ln_u{ei}")
                        stt_mm(u[:], t[:])
                        eng.tensor_copy(ln8_out[:, TOK * k:TOK * (k + 1)],
                                        u[:])
                        out_extra(k, u, eng)

            for rep in range(reps):
              for l in range(nb):
                # ---- weights: one DMA per tensor (fp8) ----
                wq_t = sbw.tile([P, KT * D], f8, tag="wq", name="wq")
                wk_t = sbw.tile([P, KT * D], f8, tag="wk", name="wk")
                wv_t = sbw.tile([P, KT * D], f8, tag="wv", name="wv")
                fc_t = sbw.tile([P, KT * D], f8, tag="fcw", name="fcw")
                w1_t = sbw.tile([P, KT * DHID], f16, tag="w1", name="w1")
                w2_t = sbw.tile([P, 8 * D], f16, tag="w2", name="w2")
                for sb_t, src, width in ((wk_t, wk_in, D), (wv_t, wv_in, D),
                                         (wq_t, wq_in, D), (fc_t, fc_in, D),
                                         (w1_t, w1_in, DHID)):
                    nc.sync.dma_start(
                        sb_t[:].rearrange("p (k c) -> p k c", c=width),
                        src[l].rearrange("(k p) c -> p k c", p=P))
                nc.sync.dma_start(
                    w2_t[:].rearrange("p (k c) -> p k c", c=D),
                    w2_in[l].rearrange("(k p) c -> p k c", p=P))
                bt = sbb.tile([P, 44], f32, tag="bias", name="bias")
                nc.sync.dma_start(bt[:], bias_in[l, :, :])

                def bap(name, idx):
                    o = BOF[name] + idx
                    return bt[:, o:o + 1]

                # ---- k^T (feature-major, fp8 at 64x), AG_K ----
                kT8 = sba.tile([P, KT * TOK], f8, tag="kT8", name="kT8")
                for hp in range(4):
                    ps = psA.tile([P, TOK], f32, tag="ps", name="ps")
                    for k in (0, 2):
                        nc.tensor.matmul(
                            ps[:], pair2(wk_t[:], k, D, P * hp, P * (hp + 1)),
                            pair2(x8w[:], k, TOK, 0, TOK),
                            start=(k == 0), stop=(k == 2), perf_mode=DR)
                    nc.scalar.activation(
                        kT8[:, TOK * hp:TOK * (hp + 1)], ps[:], AF.Identity,
                        bias=bap("bk", hp), scale=1.0)
                cc_i = dram.tile([D, TOK + VW], f8, tag="cc_i", name="cc_i")
                cc_o = dram.tile([4 * D, TOK + VW], f8, tag="cc_o",
                                 name="cc_o")
                cc_ok = cc_o[:, 0:TOK]
                cc_ov = cc_o[:, TOK:TOK + VW]
                nc.sync.dma_start(
                    cc_i[:, 0:TOK].rearrange("(hp p) c -> p hp c", p=P),
                    kT8[:].rearrange("p (hp c) -> p hp c", c=TOK))

                # ---- v (token-major, aug fp8 at 64x), AG_V ----
                for t in range(4):
                    ps = psA.tile([P, D], f32, tag="ps", name="ps")
                    for k in (0, 2):
                        nc.tensor.matmul(
                            ps[:], pair2(x8w[:], k, TOK, P * t, P * (t + 1)),
                            pair2(wv_t[:], k, D, 0, D),
                            start=(k == 0), stop=(k == 2), perf_mode=DR)
                    # one 4D-AP copy: heads (2p, 2p+1) -> aug slots (0, 2)
                    ps4 = ps[:].rearrange("p (pr w b) -> p pr w b", pr=4, w=2)
                    vo4 = v_own[:, VW * t:VW * (t + 1)].rearrange(
                        "p (pr w2 b) -> p pr w2 b", pr=4, w2=3)[:, :, 0:3:2, :]
                    nc.scalar.activation(vo4, ps4[:, :, :, :], AF.Copy,
                                         bias=0.0, scale=1.0)
                nc.sync.dma_start(
                    cc_i[:, TOK:TOK + VW].rearrange("(t p) c -> p t c", p=P),
                    v_own[:].rearrange("p (t c) -> p t c", c=VW))
                if sim1:
                    for r in range(4):
                        nc.sync.dma_start(cc_o[D * r:D * (r + 1), :],
                                          cc_i[:, :])
                else:
                    nc.gpsimd.collective_compute(
                        "AllGather", mybir.AluOpType.bypass,
                        replica_groups=GROUPS,
                        ins=[cc_i[:].opt()], outs=[cc_o[:].opt()])

                # ---- q^T fp8 (overlaps the collectives) ----
                qT8 = sba.tile([P, KT * TOK], f8, tag="qT8", name="qT8")
                for hp in range(4):
                    ps = psA.tile([P, TOK], f32, tag="ps", name="ps")
                    for k in (0, 2):
                        nc.tensor.matmul(
                            ps[:], pair2(wq_t[:], k, D, P * hp, P * (hp + 1)),
                            pair2(x8w[:], k, TOK, 0, TOK),
                            start=(k == 0), stop=(k == 2), perf_mode=DR)
                    nc.vector.tensor_scalar_add(
                        qT8[:, TOK * hp:TOK * (hp + 1)], ps[:], bap("bq", hp))

                # ---- gather-in loads ----
                for p in range(4):
                    nc.sync.dma_start(
                        ktf[p][:].rearrange("p (c w) -> p c w", w=TOK),
                        cc_ok.rearrange(
                            "(c p q) w -> c p q w", p=4, q=P)[:, p, :, :]
                        .rearrange("c q w -> q c w"))
                for cch in range(4):
                    nc.sync.dma_start(
                        va_all[:, VW * 4 * cch:VW * 4 * (cch + 1)]
                        .rearrange("p (j c) -> p j c", c=VW),
                        cc_ov[D * cch:D * (cch + 1), :]
                        .rearrange("(j p) c -> p j c", p=P))

                # ---- attention, head pairs ----
                oT8w = sba.tile([P, KT * TOK], f8, tag="oT8w", name="oT8w")
                va_r = va_all[:].rearrange("p (j c) -> p j c", c=VW)
                for hp in range(4):
                    qs = qT8[:, TOK * hp:TOK * (hp + 1)]
                    po_e = psA.tile([P, TOK], f32, tag="ps", name="ps")
                    po_o = psA.tile([P, TOK], f32, tag="ps", name="ps")
                    e_list = []
                    for g in range(8):
                        ps_e = psS.tile([P, 1024], f32, tag="ps_sc",
                                        name="ps_sc")
                        ps_o = psS.tile([P, 1024], f32, tag="ps_sc",
                                        name="ps_sc")
                        for c in range(2):
                            j = 2 * g + c
                            nc.tensor.matmul(
                                ps_e[:, TOK * c:TOK * (c + 1)],
                                ktf[hp][0:64, P * j:P * (j + 1)],
                                qs[0:64, :], start=True, stop=True)
                            nc.tensor.matmul(
                                ps_o[:, TOK * c:TOK * (c + 1)],
                                ktf[hp][64:128, P * j:P * (j + 1)],
                                qs[64:128, :], start=True, stop=True)
                        e_e = sbe.tile([P, 1024], u8, tag="e", name="e")
                        e_o = sbe.tile([P, 1024], u8, tag="e", name="e")
                        # interleave: Act every g; Pool/DVE alternate
                        nc.scalar.activation(e_e[:].bitcast(f8), ps_e[:],
                                             AF.Exp, bias=0.0, scale=SEXP)
                        if g == 0:
                            nc.scalar.activation(e_o[:].bitcast(f8), ps_o[:],
                                                 AF.Exp, bias=0.0, scale=SEXP)
                        else:
                            nc.vector.tensor_scalar(e_o[:], ps_o[:], FE_A,
                                                    FE_B, OP.mult, OP.add)
                        e_list.append((e_e, e_o))
                        if g >= 1:
                            pe_, po_ = e_list[g - 1]
                            jg = g - 1
                            nc.tensor.matmul(
                                po_e[:],
                                va_r[:, 2 * jg:2 * jg + 2,
                                     192 * hp:192 * hp + 128],
                                pe_[:].bitcast(f8).rearrange(
                                    "p (t c) -> p t c", t=2),
                                start=(jg == 0), stop=False, perf_mode=DR)
                            nc.tensor.matmul(
                                po_o[:],
                                va_r[:, 2 * jg:2 * jg + 2,
                                     192 * hp + 64:192 * hp + 192],
                                po_[:].bitcast(f8).rearrange(
                                    "p (t c) -> p t c", t=2),
                                start=(jg == 0), stop=False, perf_mode=DR)
                    pe_, po_ = e_list[7]
                    nc.tensor.matmul(
                        po_e[:], va_r[:, 14:16, 192 * hp:192 * hp + 128],
                        pe_[:].bitcast(f8).rearrange("p (t c) -> p t c", t=2),
                        start=False, stop=True, perf_mode=DR)
                    nc.tensor.matmul(
                        po_o[:], va_r[:, 14:16, 192 * hp + 64:192 * hp + 192],
                        po_[:].bitcast(f8).rearrange("p (t c) -> p t c", t=2),
                        start=False, stop=True, perf_mode=DR)
                    # normalize: denominators are rows dl; o rows ol
                    for par, po in ((0, po_e), (1, po_o)):
                        ol = slice(64 * par, 64 * par + 64)
                        dl = slice(64 * (1 - par), 64 * (1 - par) + 64)
                        dcp = sba.tile([P, TOK], f16, tag=f"dcp{par}",
                                       name=f"dcp{par}")
                        nc.scalar.activation(dcp[dl, :], po[dl, :], AF.Copy,
                                             bias=0.0, scale=1.0 / 64.0)
                        ps2 = psA.tile([P, TOK], f32, tag="ps", name="ps")
                        nc.tensor.matmul(ps2[:], ones16[dl, :], dcp[dl, :],
                                         start=True, stop=True)
                        rec = sba.tile([P, TOK], f32, tag=f"rec{par}",
                                       name=f"rec{par}")
                        nc.vector.reciprocal_approx_fast(rec[:], ps2[:])
                        nc.vector.scalar_tensor_tensor(
                            oT8w[:, TOK * hp:TOK * (hp + 1)][ol, :],
                            in0=po[ol, :], scalar=float(1.0 / WS),
                            in1=rec[ol, :], op0=OP.mult, op1=OP.mult)

                # ---- fc + residual (xmid = 64x) ----
                xmid = []
                for m in range(4):
                    ps = psA.tile([P, TOK], f32, tag="ps", name="ps")
                    for k in (0, 2):
                        nc.tensor.matmul(
                            ps[:], pair2(fc_t[:], k, D, P * m, P * (m + 1)),
                            pair2(oT8w[:], k, TOK, 0, TOK),
                            start=(k == 0), stop=(k == 2), perf_mode=DR)
                    xm = sba.tile([P, TOK], f32, tag=f"xmid{m}",
                                  name=f"xmid{m}")
                    nc.vector.scalar_tensor_tensor(
                        xm[:], in0=ps[:], scalar=bap("fcb", m), in1=xin[m][:],
                        op0=OP.add, op1=OP.add)
                    xmid.append(xm)

                # ---- LN1 -> MLP ----
                ln8w = sba.tile([P, KT * TOK], f16, tag="ln8w", name="ln8w")
                layer_norm(xmid, lambda k: bap("g1", k), ln8w, None)
                h8w = sba.tile([P, 8 * TOK], f16, tag="h8w", name="h8w")
                for m in range(8):
                    ps = psA.tile([P, TOK], f32, tag="ps", name="ps")
                    for k in range(KT):
                        nc.tensor.matmul(
                            ps[:],
                            w1_t[:, DHID * k + P * m:DHID * k + P * (m + 1)],
                            ln8w[:, TOK * k:TOK * (k + 1)],
                            start=(k == 0), stop=(k == KT - 1))
                    nc.scalar.activation(h8w[:, TOK * m:TOK * (m + 1)], ps[:],
                                         AF.Relu, bias=bap("b1", m), scale=1.0)
                xout = []
                for m in range(4):
                    ps = psA.tile([P, TOK], f32, tag="ps", name="ps")
                    for k in range(8):
                        nc.tensor.matmul(
                            ps[:], w2_t[:, D * k + P * m:D * k + P * (m + 1)],
                            h8w[:, TOK * k:TOK * (k + 1)],
                            start=(k == 0), stop=(k == 7))
                    tb = sba.tile([P, TOK], f32, tag=f"tb{m}", name=f"tb{m}")
                    nc.vector.tensor_scalar(tb[:], ps[:], 1.0 / WS,
                                            bap("b2", m), OP.mult, OP.add)
                    xo = sba.tile([P, TOK], f32, tag=f"xout{m}",
                                  name=f"xout{m}")
                    nc.gpsimd.tensor_add(xo[:], tb[:], xmid[m][:])
                    xout.append(xo)

                # ---- LN2 -> next block's x (fp8 sans bias + f32@64x) ----
                x8w = sba.tile([P, KT * TOK], f8, tag="x8w", name="x8w")
                xin = [sba.tile([P, TOK], f32, tag=f"x32_{k}",
                                name=f"x32_{k}") for k in range(KT)]
                last = (rep == reps - 1) and (l == nb - 1)

                if last:
                    def extra(k, u, eng, xin=xin):
                        eng.tensor_scalar_add(xin[k][:], u[:], bap("be2", k))
                else:
                    def extra(k, u, eng, xin=xin):
                        eng.tensor_scalar(xin[k][:], u[:], WS,
                                          bap("be2s", k), OP.mult, OP.add)
                layer_norm(xout, lambda k: bap("g2", k), x8w, extra)

            for k in range(KT):
                nc.sync.dma_start(yT_out[P * k:P * (k + 1), :], xin[k][:])

    bacc_mod.get_activation_tables = _patched
    hw_specs.get_activation_tables = _patched
    try:
        nc.compile()
    finally:
        bacc_mod.get_activation_tables = _orig_tables
        hw_specs.get_activation_tables = _orig_tables
    return nc


def _host_prep(inputs, nb):
    import ml_dtypes
    f8t = ml_dtypes.float8_e4m3fn
    qkv_w = np.asarray(inputs["qkv_w"], dtype=np.float32)[:nb]
    qkv_b = np.asarray(inputs["qkv_b"], dtype=np.float32)[:nb]
    fc_w = np.asarray(inputs["fc_w"], dtype=np.float32)[:nb]
    fc_b = np.asarray(inputs["fc_b"], dtype=np.float32)[:nb]
    w1 = np.asarray(inputs["w1"], dtype=np.float32)[:nb]
    b1 = np.asarray(inputs["b1"], dtype=np.float32)[:nb]
    w2 = np.asarray(inputs["w2"], dtype=np.float32)[:nb]
    b2 = np.asarray(inputs["b2"], dtype=np.float32)[:nb]
    g1 = np.asarray(inputs["ln1_g"], dtype=np.float32)[:nb]
    be1 = np.asarray(inputs["ln1_b"], dtype=np.float32)[:nb]
    g2 = np.asarray(inputs["ln2_g"], dtype=np.float32)[:nb]
    be2 = np.asarray(inputs["ln2_b"], dtype=np.float32)[:nb]

    idx_q = np.concatenate([np.arange(192 * h, 192 * h + 64)
                            for h in range(H)])
    idx_k = idx_q + 64
    idx_v = idx_q + 128

    def btile(b, nt):  # [nb, N] -> [nb, P, nt] with [l, p, m] = b[l, 128m+p]
        return b.reshape(nb, nt, P).transpose(0, 2, 1)

    wq_s = qkv_w[:, :, idx_q]
    wk_s = qkv_w[:, :, idx_k]
    wv_s = qkv_w[:, :, idx_v]
    # LN output biases are not applied on-device; fold them into the next
    # layer's biases. Block l>0 inputs x8 lack be2[l-1]; LN1 output lacks be1.
    dt64 = np.float64
    prev_be2 = np.concatenate(
        [np.zeros((1, D), np.float32), be2[:-1]], axis=0).astype(dt64)
    bq_eff = qkv_b[:, idx_q] + np.einsum(
        "ld,ldf->lf", prev_be2, wq_s.astype(dt64)).astype(np.float32)
    bk_eff = qkv_b[:, idx_k] + np.einsum(
        "ld,ldf->lf", prev_be2, wk_s.astype(dt64)).astype(np.float32)
    bv_eff = qkv_b[:, idx_v] + np.einsum(
        "ld,ldf->lf", prev_be2, wv_s.astype(dt64)).astype(np.float32)
    fcb_eff = fc_b + np.einsum("ld,ldf->lf", bv_eff.astype(dt64),
                               fc_w.astype(dt64)).astype(np.float32)
    b1_eff = b1 + np.einsum("ld,ldf->lf", be1.astype(dt64),
                            w1.astype(dt64)).astype(np.float32)
    biases = np.concatenate([
        btile(bq_eff * WS, 4), btile(bk_eff * WS, 4),
        btile(fcb_eff * WS, 4), btile(b2 * WS, 4), btile(g1, 4),
        btile(be1, 4), btile(g2, 4), btile(be2, 4), btile(b1_eff * WS, 8),
        btile(be2 * WS, 4)], axis=2)

    def w8(a):
        return np.ascontiguousarray(a * WS).astype(f8t)

    common = {
        "wq": w8(wq_s),
        "wk": w8(wk_s),
        "wv": w8(wv_s),
        "fcw": w8(fc_w),
        "w1": np.ascontiguousarray(w1 * WS).astype(np.float16),
        "w2": np.ascontiguousarray(w2 * WS).astype(np.float16),
        "biases": np.ascontiguousarray(biases),
    }
    X = np.asarray(inputs["X"], dtype=np.float32)
    in_maps = []
    for c in range(N_CORES):
        b, r = c // 4, c % 4
        xT = np.ascontiguousarray(X[b, TOK * r:TOK * (r + 1), :].T) * WS
        in_maps.append({"xT": xT, **common})
    return in_maps


def get_nc(nb=NB, reps=1):
    key = (nb, reps)
    if key not in _CACHE:
        _CACHE[key] = _build(nb, reps)
    return _CACHE[key]


def kernel(**inputs):
    from concourse.bass_utils import run_bass_kernel_spmd

    nb = NB
    nc = get_nc(nb)
    in_maps = _host_prep(inputs, nb)
    res = run_bass_kernel_spmd(nc, in_maps, list(range(N_CORES)))
    Y = np.zeros((B, S, D), dtype=np.float32)
    for c in range(N_CORES):
        b, r = c // 4, c % 4
        Y[b, TOK * r:TOK * (r + 1), :] = res.results[c]["yT"].T
    return Y


# revision 12
# speedup vs baseline: 1.0520x; 1.0276x over previous
"""12-block transformer encoder (B=2, S=2048, D=512, H=8, DHID=1024) on 8 trn2 cores.

Sequence-parallel: core c owns batch c//4, tokens 512*(c%4)..+512. Weights
replicated, fp8e4m3 at 64x scale (escapes e4m3 denormals; folds undone in
existing scalar slots). Residual stream f32 at 64x. Projections + attn@V run
fp8 DoubleRow (2 contraction chunks per matmul, 0.5 cyc/row). Scores fp8
1 cyc/row. Softmax exp split across engines: exact Exp on Activation, bit-trick
fast-exp (tensor_scalar -> uint8 bits viewed as fp8e4) on Pool and DVE,
interleaved per score-group so all three engines run concurrently.
LayerNorm rstd = Exp(-0.5*Ln(D*var)+lnC) on Activation; a get_activation_tables
patch during build makes the table pass pick the single set containing
exp+ln+relu, so there is exactly one table load. LN output biases are folded
into the next layer's biases host-side. K^T and aug-V AllGathered in fp8 as
two collectives (K first so scores start sooner; ones columns ride with V so
the softmax denominator falls out of the attn@V matmul).
"""
import sys
import numpy as np

for _p in ("/opt/trn_rl_repo", "/root/.axon_site/_ro/trn_rl_repo"):
    if _p not in sys.path:
        sys.path.insert(0, _p)

P = 128
B, S, D = 2, 2048, 512
H, DH, DHID = 8, 64, 1024
NB = 12
TOK = 512            # tokens per core
KT = D // P          # 4 contraction chunks over D
N_CORES = 8
GROUPS = [[0, 1, 2, 3], [4, 5, 6, 7]]
VW = 768             # aug-v row width per token ptile (4 pairs x 192)
WS = 64.0            # weight scale
LOG2E = 1.4426950408889634
SPLIT_AG = False

_CACHE = {}


def _build(nb, reps=1):
    import os
    import concourse.bass as bass
    import concourse.mybir as mybir
    import concourse.tile as tile
    from concourse import bacc
    import concourse.hw_specs as hw_specs
    import concourse.bacc as bacc_mod

    f32 = mybir.dt.float32
    f16 = mybir.dt.float16
    f8 = mybir.dt.float8e4
    u8 = mybir.dt.uint8
    AF = mybir.ActivationFunctionType
    OP = mybir.AluOpType
    DR = mybir.MatmulPerfMode.DoubleRow

    # Steer the act-table pass to the one set holding exp+ln+relu so the
    # kernel needs a single table load. Runtime-correct: we only narrow the
    # claimed contents of the other sets; ids stay the json indices.
    _orig_tables = hw_specs.get_activation_tables

    def _patched(arch):
        t = _orig_tables(arch)
        for name, s in t.items():
            if name != "natural_log_exp_and_others":
                s.discard(AF.Exp)
                s.discard(AF.Ln)
                s.discard(AF.Relu)
        return t

    sim1 = os.environ.get("KSIM") in ("1", "2")
    sim_cheap = os.environ.get("KSIM") == "2"
    nc = bacc.Bacc("TRN2", target_bir_lowering=False, debug=False,
                   num_devices=(1 if sim1 else N_CORES))

    xT_in = nc.declare_dram_parameter("xT", [D, TOK], f32, isOutput=False)
    wq_in = nc.declare_dram_parameter("wq", [nb, D, D], f8, isOutput=False)
    wk_in = nc.declare_dram_parameter("wk", [nb, D, D], f8, isOutput=False)
    wv_in = nc.declare_dram_parameter("wv", [nb, D, D], f8, isOutput=False)
    fc_in = nc.declare_dram_parameter("fcw", [nb, D, D], f8, isOutput=False)
    w1_in = nc.declare_dram_parameter("w1", [nb, D, DHID], f16, isOutput=False)
    w2_in = nc.declare_dram_parameter("w2", [nb, DHID, D], f16, isOutput=False)
    bias_in = nc.declare_dram_parameter("biases", [nb, P, 44], f32,
                                        isOutput=False)
    yT_out = nc.declare_dram_parameter("yT", [D, TOK], f32, isOutput=True)

    inv_sqrt_d = float(1.0 / np.sqrt(D))
    SEXP = inv_sqrt_d / (WS * WS)     # exp arg = psum * SEXP
    FE_A = LOG2E * 8.0 * SEXP         # fast-exp bits = psum*FE_A + FE_B
    FE_B = 7.0 * 8.0 - 0.58
    BOF = {"bq": 0, "bk": 4, "fcb": 8, "b2": 12, "g1": 16, "be1": 20,
           "g2": 24, "be2": 28, "b1": 32, "be2s": 40}

    with tile.TileContext(nc) as tc:
        with tc.tile_pool(name="w", bufs=2) as sbw, \
             tc.tile_pool(name="bias", bufs=2) as sbb, \
             tc.tile_pool(name="act", bufs=1) as sba, \
             tc.tile_pool(name="e", bufs=8) as sbe, \
             tc.tile_pool(name="pers", bufs=1) as sbp, \
             tc.tile_pool(name="psS", bufs=2, space="PSUM") as psS, \
             tc.tile_pool(name="psA", bufs=4, space="PSUM") as psA, \
             tc.tile_pool(name="dram", bufs=2, space="DRAM") as dram:

            ones16 = sbp.tile([P, P], f16, tag="ones16", name="ones16")
            nc.vector.memset(ones16[:], 1.0)
            lnC = sbp.tile([P, 1], f32, tag="lnC", name="lnC")
            nc.vector.memset(lnC[:], float(np.log(np.sqrt(D) / WS)))
            ones8 = sbp.tile([P, 2 * P], f8, tag="ones8", name="ones8")
            nc.vector.memset(ones8[:], 1.0)
            on2 = ones8[:].rearrange("p (t m) -> p t m", t=2)
            ktf = [sbp.tile([P, S], f8, tag=f"ktf{p}", name=f"ktf{p}")
                   for p in range(4)]
            # all 16 key-ptile aug-v tiles in one tensor: [P, 16*768] fp8
            va_all = sbp.tile([P, 16 * VW], f8, tag="va", name="va")
            # own-token plain v staging (head-major); aug ones only in va_all
            v8 = sbp.tile([P, KT * D], f8, tag="v8", name="v8")
            oc = va_all[:].rearrange("p (a b) -> p a b", b=64)
            nc.vector.memset(oc[:, 1:oc.shape[1]:3, :], 1.0)

            # residual stream (64x) + fp8 copy
            xin = [sba.tile([P, TOK], f32, tag=f"x32_{k}", name=f"x32_{k}")
                   for k in range(KT)]
            x8w = sba.tile([P, KT * TOK], f8, tag="x8w", name="x8w")
            for k in range(KT):
                nc.sync.dma_start(xin[k][:], xT_in[P * k:P * (k + 1), :])
                nc.vector.tensor_scalar_mul(
                    x8w[:, TOK * k:TOK * (k + 1)], xin[k][:], 1.0 / WS)

            def pair2(ap_wide, k, width, lo, hi):
                """[P, 2, hi-lo] view: chunks k,k+1 of ap_wide, cols lo:hi."""
                r = ap_wide.rearrange("p (k c) -> p k c", c=width)
                return r[:, k:k + 2, lo:hi]

            def layer_norm(x4, g_ap, ln8_out, out_extra):
                """x4: 4 f32 [P,TOK] tiles at 64x. Writes normalized*g (no
                +bias: folded downstream) as fp8 into ln8_out (wide); calls
                out_extra(k, u32, eng) for extra outputs (u32 f32)."""
                m8 = sba.tile([P, KT * TOK], f8, tag="ln_m8", name="ln_m8")
                sq8 = sba.tile([P, KT * TOK], f8, tag="ln_sq8", name="ln_sq8")
                for k in range(KT):
                    cs = slice(TOK * k, TOK * (k + 1))
                    eng = (nc.gpsimd, nc.vector)[k % 2]
                    eng.tensor_scalar_mul(m8[:, cs], x4[k][:], 1.0 / WS)
                    eng.tensor_mul(sq8[:, cs], m8[:, cs], m8[:, cs])
                psum_s = psA.tile([P, TOK], f32, tag="ps", name="ps")
                psum_q = psA.tile([P, TOK], f32, tag="ps", name="ps")
                for k in (0, 2):
                    nc.tensor.matmul(psum_s[:], on2,
                                     pair2(m8[:], k, TOK, 0, TOK),
                                     start=(k == 0), stop=(k == 2),
                                     perf_mode=DR)
                    nc.tensor.matmul(psum_q[:], on2,
                                     pair2(sq8[:], k, TOK, 0, TOK),
                                     start=(k == 0), stop=(k == 2),
                                     perf_mode=DR)
                a = sba.tile([P, TOK], f32, tag="ln_a", name="ln_a")
                nc.vector.tensor_scalar_mul(a[:], psum_s[:], WS / D)
                a2 = sba.tile([P, TOK], f32, tag="ln_a2", name="ln_a2")
                nc.gpsimd.tensor_mul(a2[:], a[:], a[:])
                pt = sba.tile([P, TOK], f32, tag="ln_pt", name="ln_pt")
                nc.vector.scalar_tensor_tensor(
                    pt[:], in0=a2[:], scalar=float(-D / (WS * WS)),
                    in1=psum_q[:], op0=OP.mult, op1=OP.add)
                # rstd*C = exp(-0.5*ln(pt) + ln(C)), C = sqrt(D)/WS
                lnv = sba.tile([P, TOK], f32, tag="ln_lnv", name="ln_lnv")
                nc.scalar.activation(lnv[:], pt[:], AF.Ln, bias=0.0, scale=1.0)
                rstd = sba.tile([P, TOK], f32, tag="ln_rstd", name="ln_rstd")
                nc.scalar.activation(rstd[:], lnv[:], AF.Exp,
                                     bias=lnC[:, 0:1], scale=-0.5)
                for k in range(KT):
                    ei = k % 2
                    eng = (nc.vector, nc.gpsimd)[ei]
                    t = sba.tile([P, TOK], f32, tag=f"ln_t{ei}",
                                 name=f"ln_t{ei}")
                    eng.tensor_sub(t[:], x4[k][:], a[:])

                    def stt_mm(out_ap, tin):
                        # (t * g) * rstd; Pool lacks scalar_tensor_tensor
                        if ei == 0:
                            eng.scalar_tensor_tensor(
                                out_ap, in0=tin, scalar=g_ap(k), in1=rstd[:],
                                op0=OP.mult, op1=OP.mult)
                        else:
                            w = sba.tile([P, TOK], f32, tag="ln_w1",
                                         name="ln_w1")
                            eng.tensor_scalar_mul(w[:], tin, g_ap(k))
                            eng.tensor_mul(out_ap, w[:], rstd[:])

                    if out_extra is None:
                        stt_mm(ln8_out[:, TOK * k:TOK * (k + 1)], t[:])
                    else:
                        u = sba.tile([P, TOK], f32, tag=f"ln_u{ei}",
                                     name=f"ln_u{ei}")
                        stt_mm(u[:], t[:])
                        eng.tensor_copy(ln8_out[:, TOK * k:TOK * (k + 1)],
                                        u[:])
                        out_extra(k, u, eng)

            for rep in range(reps):
              for l in range(nb):
                # ---- weights: one DMA per tensor (fp8) ----
                wq_t = sbw.tile([P, KT * D], f8, tag="wq", name="wq")
                wk_t = sbw.tile([P, KT * D], f8, tag="wk", name="wk")
                wv_t = sbw.tile([P, KT * D], f8, tag="wv", name="wv")
                fc_t = sbw.tile([P, KT * D], f8, tag="fcw", name="fcw")
                w1_t = sbw.tile([P, KT * DHID], f16, tag="w1", name="w1")
                w2_t = sbw.tile([P, 8 * D], f16, tag="w2", name="w2")
                for sb_t, src, width in ((wk_t, wk_in, D), (wv_t, wv_in, D),
                                         (wq_t, wq_in, D), (fc_t, fc_in, D),
                                         (w1_t, w1_in, DHID)):
                    nc.sync.dma_start(
                        sb_t[:].rearrange("p (k c) -> p k c", c=width),
                        src[l].rearrange("(k p) c -> p k c", p=P))
                nc.sync.dma_start(
                    w2_t[:].rearrange("p (k c) -> p k c", c=D),
                    w2_in[l].rearrange("(k p) c -> p k c", p=P))
                bt = sbb.tile([P, 44], f32, tag="bias", name="bias")
                nc.sync.dma_start(bt[:], bias_in[l, :, :])

                def bap(name, idx):
                    o = BOF[name] + idx
                    return bt[:, o:o + 1]

                # ---- k^T (feature-major, fp8 at 64x), AG_K ----
                kT8 = sba.tile([P, KT * TOK], f8, tag="kT8", name="kT8")
                for hp in range(4):
                    ps = psA.tile([P, TOK], f32, tag="ps", name="ps")
                    for k in (0, 2):
                        nc.tensor.matmul(
                            ps[:], pair2(wk_t[:], k, D, P * hp, P * (hp + 1)),
                            pair2(x8w[:], k, TOK, 0, TOK),
                            start=(k == 0), stop=(k == 2), perf_mode=DR)
                    nc.scalar.activation(
                        kT8[:, TOK * hp:TOK * (hp + 1)], ps[:], AF.Identity,
                        bias=bap("bk", hp), scale=1.0)
                cc_i = dram.tile([D, TOK + D], f8, tag="cc_i", name="cc_i")
                cc_o = dram.tile([4 * D, TOK + D], f8, tag="cc_o",
                                 name="cc_o")
                cc_ok = cc_o[:, 0:TOK]
                nc.sync.dma_start(
                    cc_i[:, 0:TOK].rearrange("(hp p) c -> p hp c", p=P),
                    kT8[:].rearrange("p (hp c) -> p hp c", c=TOK))

                # ---- v (token-major, aug fp8 at 64x), AG_V ----
                for t in range(4):
                    ps = psA.tile([P, D], f32, tag="ps", name="ps")
                    for k in (0, 2):
                        nc.tensor.matmul(
                            ps[:], pair2(x8w[:], k, TOK, P * t, P * (t + 1)),
                            pair2(wv_t[:], k, D, 0, D),
                            start=(k == 0), stop=(k == 2), perf_mode=DR)
                    nc.vector.tensor_copy(v8[:, D * t:D * (t + 1)], ps[:])
                nc.sync.dma_start(
                    cc_i[:, TOK:TOK + D].rearrange("(t p) c -> p t c", p=P),
                    v8[:].rearrange("p (t c) -> p t c", c=D))
                if sim1:
                    for r in range(4):
                        nc.sync.dma_start(cc_o[D * r:D * (r + 1), :],
                                          cc_i[:, :])
                else:
                    nc.gpsimd.collective_compute(
                        "AllGather", mybir.AluOpType.bypass,
                        replica_groups=GROUPS,
                        ins=[cc_i[:].opt()], outs=[cc_o[:].opt()])

                # ---- q^T fp8 (overlaps the collectives) ----
                qT8 = sba.tile([P, KT * TOK], f8, tag="qT8", name="qT8")
                for hp in range(4):
                    ps = psA.tile([P, TOK], f32, tag="ps", name="ps")
                    for k in (0, 2):
                        nc.tensor.matmul(
                            ps[:], pair2(wq_t[:], k, D, P * hp, P * (hp + 1)),
                            pair2(x8w[:], k, TOK, 0, TOK),
                            start=(k == 0), stop=(k == 2), perf_mode=DR)
                    nc.vector.tensor_scalar_add(
                        qT8[:, TOK * hp:TOK * (hp + 1)], ps[:], bap("bq", hp))

                # ---- gather-in loads ----
                for p in range(4):
                    nc.sync.dma_start(
                        ktf[p][:].rearrange("p (c w) -> p c w", w=TOK),
                        cc_ok.rearrange(
                            "(c p q) w -> c p q w", p=4, q=P)[:, p, :, :]
                        .rearrange("c q w -> q c w"))
                for cch in range(4):
                    for j in range(4):
                        jj = 4 * cch + j
                        src = cc_o[D * cch + P * j:D * cch + P * (j + 1),
                                   TOK:TOK + D]
                        sr = src.rearrange("p (pr w b) -> p pr w b",
                                           pr=4, w=2)
                        dst = va_all[:, VW * jj:VW * (jj + 1)].rearrange(
                            "p (pr w2 b) -> p pr w2 b", pr=4, w2=3)
                        for w in range(2):
                            nc.sync.dma_start(dst[:, :, 2 * w, :],
                                              sr[:, :, w, :])

                # ---- attention, head pairs ----
                oT8w = sba.tile([P, KT * TOK], f8, tag="oT8w", name="oT8w")
                va_r = va_all[:].rearrange("p (j c) -> p j c", c=VW)
                for hp in range(4):
                    qs = qT8[:, TOK * hp:TOK * (hp + 1)]
                    po_e = psA.tile([P, TOK], f32, tag="ps", name="ps")
                    po_o = psA.tile([P, TOK], f32, tag="ps", name="ps")
                    e_list = []
                    for g in range(8):
                        ps_e = psS.tile([P, 1024], f32, tag="ps_sc",
                                        name="ps_sc")
                        ps_o = psS.tile([P, 1024], f32, tag="ps_sc",
                                        name="ps_sc")
                        for c in range(2):
                            j = 2 * g + c
                            nc.tensor.matmul(
                                ps_e[:, TOK * c:TOK * (c + 1)],
                                ktf[hp][0:64, P * j:P * (j + 1)],
                                qs[0:64, :], start=True, stop=True)
                            nc.tensor.matmul(
                                ps_o[:, TOK * c:TOK * (c + 1)],
                                ktf[hp][64:128, P * j:P * (j + 1)],
                                qs[64:128, :], start=True, stop=True)
                        e_e = sbe.tile([P, 1024], u8, tag="e", name="e")
                        e_o = sbe.tile([P, 1024], u8, tag="e", name="e")
                        # interleave: Act every g; Pool/DVE alternate
                        nc.scalar.activation(e_e[:].bitcast(f8), ps_e[:],
                                             AF.Exp, bias=0.0, scale=SEXP)
                        if g == 0:
                            nc.scalar.activation(e_o[:].bitcast(f8), ps_o[:],
                                                 AF.Exp, bias=0.0, scale=SEXP)
                        else:
                            nc.vector.tensor_scalar(e_o[:], ps_o[:], FE_A,
                                                    FE_B, OP.mult, OP.add)
                        e_list.append((e_e, e_o))
                        if g >= 1:
                            pe_, po_ = e_list[g - 1]
                            jg = g - 1
                            nc.tensor.matmul(
                                po_e[:],
                                va_r[:, 2 * jg:2 * jg + 2,
                                     192 * hp:192 * hp + 128],
                                pe_[:].bitcast(f8).rearrange(
                                    "p (t c) -> p t c", t=2),
                                start=(jg == 0), stop=False, perf_mode=DR)
                            nc.tensor.matmul(
                                po_o[:],
                                va_r[:, 2 * jg:2 * jg + 2,
                                     192 * hp + 64:192 * hp + 192],
                                po_[:].bitcast(f8).rearrange(
                                    "p (t c) -> p t c", t=2),
                                start=(jg == 0), stop=False, perf_mode=DR)
                    pe_, po_ = e_list[7]
                    nc.tensor.matmul(
                        po_e[:], va_r[:, 14:16, 192 * hp:192 * hp + 128],
                        pe_[:].bitcast(f8).rearrange("p (t c) -> p t c", t=2),
                        start=False, stop=True, perf_mode=DR)
                    nc.tensor.matmul(
                        po_o[:], va_r[:, 14:16, 192 * hp + 64:192 * hp + 192],
                        po_[:].bitcast(f8).rearrange("p (t c) -> p t c", t=2),
                        start=False, stop=True, perf_mode=DR)
                    # normalize: denominators are rows dl; o rows ol
                    for par, po in ((0, po_e), (1, po_o)):
                        ol = slice(64 * par, 64 * par + 64)
                        dl = slice(64 * (1 - par), 64 * (1 - par) + 64)
                        dcp = sba.tile([P, TOK], f16, tag=f"dcp{par}",
                                       name=f"dcp{par}")
                        nc.scalar.activation(dcp[dl, :], po[dl, :], AF.Copy,
                                             bias=0.0, scale=1.0 / 64.0)
                        ps2 = psA.tile([P, TOK], f32, tag="ps", name="ps")
                        nc.tensor.matmul(ps2[:], ones16[dl, :], dcp[dl, :],
                                         start=True, stop=True)
                        rec = sba.tile([P, TOK], f32, tag=f"rec{par}",
                                       name=f"rec{par}")
                        nc.vector.reciprocal_approx_fast(rec[:], ps2[:])
                        nc.vector.scalar_tensor_tensor(
                            oT8w[:, TOK * hp:TOK * (hp + 1)][ol, :],
                            in0=po[ol, :], scalar=float(1.0 / WS),
                            in1=rec[ol, :], op0=OP.mult, op1=OP.mult)

                # ---- fc + residual (xmid = 64x) ----
                xmid = []
                for m in range(4):
                    ps = psA.tile([P, TOK], f32, tag="ps", name="ps")
                    for k in (0, 2):
                        nc.tensor.matmul(
                            ps[:], pair2(fc_t[:], k, D, P * m, P * (m + 1)),
                            pair2(oT8w[:], k, TOK, 0, TOK),
                            start=(k == 0), stop=(k == 2), perf_mode=DR)
                    xm = sba.tile([P, TOK], f32, tag=f"xmid{m}",
                                  name=f"xmid{m}")
                    nc.vector.scalar_tensor_tensor(
                        xm[:], in0=ps[:], scalar=bap("fcb", m), in1=xin[m][:],
                        op0=OP.add, op1=OP.add)
                    xmid.append(xm)

                # ---- LN1 -> MLP ----
                ln8w = sba.tile([P, KT * TOK], f16, tag="ln8w", name="ln8w")
                layer_norm(xmid, lambda k: bap("g1", k), ln8w, None)
                h8w = sba.tile([P, 8 * TOK], f16, tag="h8w", name="h8w")
                for m in range(8):
                    ps = psA.tile([P, TOK], f32, tag="ps", name="ps")
                    for k in range(KT):
                        nc.tensor.matmul(
                            ps[:],
                            w1_t[:, DHID * k + P * m:DHID * k + P * (m + 1)],
                            ln8w[:, TOK * k:TOK * (k + 1)],
                            start=(k == 0), stop=(k == KT - 1))
                    nc.scalar.activation(h8w[:, TOK * m:TOK * (m + 1)], ps[:],
                                         AF.Relu, bias=bap("b1", m), scale=1.0)
                xout = []
                for m in range(4):
                    ps = psA.tile([P, TOK], f32, tag="ps", name="ps")
                    for k in range(8):
                        nc.tensor.matmul(
                            ps[:], w2_t[:, D * k + P * m:D * k + P * (m + 1)],
                            h8w[:, TOK * k:TOK * (k + 1)],
                            start=(k == 0), stop=(k == 7))
                    tb = sba.tile([P, TOK], f32, tag=f"tb{m}", name=f"tb{m}")
                    nc.vector.tensor_scalar(tb[:], ps[:], 1.0 / WS,
                                            bap("b2", m), OP.mult, OP.add)
                    xo = sba.tile([P, TOK], f32, tag=f"xout{m}",
                                  name=f"xout{m}")
                    nc.gpsimd.tensor_add(xo[:], tb[:], xmid[m][:])
                    xout.append(xo)

                # ---- LN2 -> next block's x (fp8 sans bias + f32@64x) ----
                x8w = sba.tile([P, KT * TOK], f8, tag="x8w", name="x8w")
                xin = [sba.tile([P, TOK], f32, tag=f"x32_{k}",
                                name=f"x32_{k}") for k in range(KT)]
                last = (rep == reps - 1) and (l == nb - 1)

                if last:
                    def extra(k, u, eng, xin=xin):
                        eng.tensor_scalar_add(xin[k][:], u[:], bap("be2", k))
                else:
                    def extra(k, u, eng, xin=xin):
                        eng.tensor_scalar(xin[k][:], u[:], WS,
                                          bap("be2s", k), OP.mult, OP.add)
                layer_norm(xout, lambda k: bap("g2", k), x8w, extra)

            for k in range(KT):
                nc.sync.dma_start(yT_out[P * k:P * (k + 1), :], xin[k][:])

    bacc_mod.get_activation_tables = _patched
    hw_specs.get_activation_tables = _patched
    try:
        nc.compile()
    finally:
        bacc_mod.get_activation_tables = _orig_tables
        hw_specs.get_activation_tables = _orig_tables
    return nc


def _host_prep(inputs, nb):
    import ml_dtypes
    f8t = ml_dtypes.float8_e4m3fn
    qkv_w = np.asarray(inputs["qkv_w"], dtype=np.float32)[:nb]
    qkv_b = np.asarray(inputs["qkv_b"], dtype=np.float32)[:nb]
    fc_w = np.asarray(inputs["fc_w"], dtype=np.float32)[:nb]
    fc_b = np.asarray(inputs["fc_b"], dtype=np.float32)[:nb]
    w1 = np.asarray(inputs["w1"], dtype=np.float32)[:nb]
    b1 = np.asarray(inputs["b1"], dtype=np.float32)[:nb]
    w2 = np.asarray(inputs["w2"], dtype=np.float32)[:nb]
    b2 = np.asarray(inputs["b2"], dtype=np.float32)[:nb]
    g1 = np.asarray(inputs["ln1_g"], dtype=np.float32)[:nb]
    be1 = np.asarray(inputs["ln1_b"], dtype=np.float32)[:nb]
    g2 = np.asarray(inputs["ln2_g"], dtype=np.float32)[:nb]
    be2 = np.asarray(inputs["ln2_b"], dtype=np.float32)[:nb]

    idx_q = np.concatenate([np.arange(192 * h, 192 * h + 64)
                            for h in range(H)])
    idx_k = idx_q + 64
    idx_v = idx_q + 128

    def btile(b, nt):  # [nb, N] -> [nb, P, nt] with [l, p, m] = b[l, 128m+p]
        return b.reshape(nb, nt, P).transpose(0, 2, 1)

    wq_s = qkv_w[:, :, idx_q]
    wk_s = qkv_w[:, :, idx_k]
    wv_s = qkv_w[:, :, idx_v]
    # LN output biases are not applied on-device; fold them into the next
    # layer's biases. Block l>0 inputs x8 lack be2[l-1]; LN1 output lacks be1.
    dt64 = np.float64
    prev_be2 = np.concatenate(
        [np.zeros((1, D), np.float32), be2[:-1]], axis=0).astype(dt64)
    bq_eff = qkv_b[:, idx_q] + np.einsum(
        "ld,ldf->lf", prev_be2, wq_s.astype(dt64)).astype(np.float32)
    bk_eff = qkv_b[:, idx_k] + np.einsum(
        "ld,ldf->lf", prev_be2, wk_s.astype(dt64)).astype(np.float32)
    bv_eff = qkv_b[:, idx_v] + np.einsum(
        "ld,ldf->lf", prev_be2, wv_s.astype(dt64)).astype(np.float32)
    fcb_eff = fc_b + np.einsum("ld,ldf->lf", bv_eff.astype(dt64),
                               fc_w.astype(dt64)).astype(np.float32)
    b1_eff = b1 + np.einsum("ld,ldf->lf", be1.astype(dt64),
                            w1.astype(dt64)).astype(np.float32)
    biases = np.concatenate([
        btile(bq_eff * WS, 4), btile(bk_eff * WS, 4),
        btile(fcb_eff * WS, 4), btile(b2 * WS, 4), btile(g1, 4),
        btile(be1, 4), btile(g2, 4), btile(be2, 4), btile(b1_eff * WS, 8),
        btile(be2 * WS, 4)], axis=2)

    def w8(a):
        return np.ascontiguousarray(a * WS).astype(f8t)

    common = {
        "wq": w8(wq_s),
        "wk": w8(wk_s),
        "wv": w8(wv_s),
        "fcw": w8(fc_w),
        "w1": np.ascontiguousarray(w1 * WS).astype(np.float16),
        "w2": np.ascontiguousarray(w2 * WS).astype(np.float16),
        "biases": np.ascontiguousarray(biases),
    }
    X = np.asarray(inputs["X"], dtype=np.float32)
    in_maps = []
    for c in range(N_CORES):
        b, r = c // 4, c % 4
        xT = np.ascontiguousarray(X[b, TOK * r:TOK * (r + 1), :].T) * WS
        in_maps.append({"xT": xT, **common})
    return in_maps


def get_nc(nb=NB, reps=1):
    key = (nb, reps)
    if key not in _CACHE:
        _CACHE[key] = _build(nb, reps)
    return _CACHE[key]


def kernel(**inputs):
    from concourse.bass_utils import run_bass_kernel_spmd

    nb = NB
    nc = get_nc(nb)
    in_maps = _host_prep(inputs, nb)
    res = run_bass_kernel_spmd(nc, in_maps, list(range(N_CORES)))
    Y = np.zeros((B, S, D), dtype=np.float32)
    for c in range(N_CORES):
        b, r = c // 4, c % 4
        Y[b, TOK * r:TOK * (r + 1), :] = res.results[c]["yT"].T
    return Y


# revision 13
# speedup vs baseline: 1.0631x; 1.0105x over previous
"""12-block transformer encoder (B=2, S=2048, D=512, H=8, DHID=1024) on 8 trn2 cores.

Sequence-parallel: core c owns batch c//4, tokens 512*(c%4)..+512. Weights
replicated, fp8e4m3 at 64x scale (escapes e4m3 denormals; folds undone in
existing scalar slots). Residual stream f32 at 64x. Projections + attn@V run
fp8 DoubleRow (2 contraction chunks per matmul, 0.5 cyc/row). Scores fp8
1 cyc/row. Softmax exp split across engines: exact Exp on Activation, bit-trick
fast-exp (tensor_scalar -> uint8 bits viewed as fp8e4) on Pool and DVE,
interleaved per score-group so all three engines run concurrently.
MLP (w1/w2) stays fp16: fp8 quantization noise there does not average down
and was the dominant error term. LayerNorm rstd = Exp(-0.5*Ln(D*var)+lnC) on
Activation; a get_activation_tables patch during build makes the table pass
pick the single set containing exp+ln+relu, so there is exactly one table
load. LN output biases are folded into the next layer's biases host-side.
K^T and plain V AllGathered in fp8 as one collective; the gather-in DMAs
spread V into the aug layout (persistent ones columns in SBUF supply the
softmax denominator through the attn@V DoubleRow matmul).
"""
import sys
import numpy as np

for _p in ("/opt/trn_rl_repo", "/root/.axon_site/_ro/trn_rl_repo"):
    if _p not in sys.path:
        sys.path.insert(0, _p)

P = 128
B, S, D = 2, 2048, 512
H, DH, DHID = 8, 64, 1024
NB = 12
TOK = 512            # tokens per core
KT = D // P          # 4 contraction chunks over D
N_CORES = 8
GROUPS = [[0, 1, 2, 3], [4, 5, 6, 7]]
VW = 768             # aug-v row width per token ptile (4 pairs x 192)
WS = 64.0            # weight scale
LOG2E = 1.4426950408889634
SPLIT_AG = False

_CACHE = {}


def _build(nb, reps=1):
    import os
    import concourse.bass as bass
    import concourse.mybir as mybir
    import concourse.tile as tile
    from concourse import bacc
    import concourse.hw_specs as hw_specs
    import concourse.bacc as bacc_mod

    f32 = mybir.dt.float32
    f16 = mybir.dt.float16
    f8 = mybir.dt.float8e4
    u8 = mybir.dt.uint8
    AF = mybir.ActivationFunctionType
    OP = mybir.AluOpType
    DR = mybir.MatmulPerfMode.DoubleRow

    # Steer the act-table pass to the one set holding exp+ln+relu so the
    # kernel needs a single table load. Runtime-correct: we only narrow the
    # claimed contents of the other sets; ids stay the json indices.
    _orig_tables = hw_specs.get_activation_tables

    def _patched(arch):
        t = _orig_tables(arch)
        for name, s in t.items():
            if name != "natural_log_exp_and_others":
                s.discard(AF.Exp)
                s.discard(AF.Ln)
                s.discard(AF.Relu)
        return t

    sim1 = os.environ.get("KSIM") in ("1", "2")
    sim_cheap = os.environ.get("KSIM") == "2"
    nc = bacc.Bacc("TRN2", target_bir_lowering=False, debug=False,
                   num_devices=(1 if sim1 else N_CORES))

    xT_in = nc.declare_dram_parameter("xT", [D, TOK], f32, isOutput=False)
    wq_in = nc.declare_dram_parameter("wq", [nb, D, D], f8, isOutput=False)
    wk_in = nc.declare_dram_parameter("wk", [nb, D, D], f8, isOutput=False)
    wv_in = nc.declare_dram_parameter("wv", [nb, D, D], f8, isOutput=False)
    fc_in = nc.declare_dram_parameter("fcw", [nb, D, D], f8, isOutput=False)
    w1_in = nc.declare_dram_parameter("w1", [nb, D, DHID], f16, isOutput=False)
    w2_in = nc.declare_dram_parameter("w2", [nb, DHID, D], f16, isOutput=False)
    bias_in = nc.declare_dram_parameter("biases", [nb, P, 44], f32,
                                        isOutput=False)
    yT_out = nc.declare_dram_parameter("yT", [D, TOK], f32, isOutput=True)

    inv_sqrt_d = float(1.0 / np.sqrt(D))
    SEXP = inv_sqrt_d / (WS * WS)     # exp arg = psum * SEXP
    FE_A = LOG2E * 8.0 * SEXP         # fast-exp bits = psum*FE_A + FE_B
    FE_B = 7.0 * 8.0 - 0.58
    BOF = {"bq": 0, "bk": 4, "fcb": 8, "b2": 12, "g1": 16, "be1": 20,
           "g2": 24, "be2": 28, "b1": 32, "be2s": 40}

    with tile.TileContext(nc) as tc:
        with tc.tile_pool(name="w", bufs=2) as sbw, \
             tc.tile_pool(name="bias", bufs=2) as sbb, \
             tc.tile_pool(name="act", bufs=1) as sba, \
             tc.tile_pool(name="e", bufs=8) as sbe, \
             tc.tile_pool(name="pers", bufs=1) as sbp, \
             tc.tile_pool(name="psS", bufs=2, space="PSUM") as psS, \
             tc.tile_pool(name="psA", bufs=4, space="PSUM") as psA, \
             tc.tile_pool(name="dram", bufs=2, space="DRAM") as dram:

            ones16 = sbp.tile([P, P], f16, tag="ones16", name="ones16")
            nc.vector.memset(ones16[:], 1.0)
            lnC = sbp.tile([P, 1], f32, tag="lnC", name="lnC")
            nc.vector.memset(lnC[:], float(np.log(np.sqrt(D) / WS)))
            ones8 = sbp.tile([P, 2 * P], f8, tag="ones8", name="ones8")
            nc.vector.memset(ones8[:], 1.0)
            on2 = ones8[:].rearrange("p (t m) -> p t m", t=2)
            ktf = [sbp.tile([P, S], f8, tag=f"ktf{p}", name=f"ktf{p}")
                   for p in range(4)]
            # all 16 key-ptile aug-v tiles in one tensor: [P, 16*768] fp8
            va_all = sbp.tile([P, 16 * VW], f8, tag="va", name="va")
            # own-token plain v staging (head-major); aug ones only in va_all
            v8 = sbp.tile([P, KT * D], f8, tag="v8", name="v8")
            oc = va_all[:].rearrange("p (a b) -> p a b", b=64)
            nc.vector.memset(oc[:, 1:oc.shape[1]:3, :], 1.0)

            # residual stream (64x) + fp8 copy
            xin = [sba.tile([P, TOK], f32, tag=f"x32_{k}", name=f"x32_{k}")
                   for k in range(KT)]
            x8w = sba.tile([P, KT * TOK], f8, tag="x8w", name="x8w")
            for k in range(KT):
                nc.sync.dma_start(xin[k][:], xT_in[P * k:P * (k + 1), :])
                nc.vector.tensor_scalar_mul(
                    x8w[:, TOK * k:TOK * (k + 1)], xin[k][:], 1.0 / WS)

            def pair2(ap_wide, k, width, lo, hi):
                """[P, 2, hi-lo] view: chunks k,k+1 of ap_wide, cols lo:hi."""
                r = ap_wide.rearrange("p (k c) -> p k c", c=width)
                return r[:, k:k + 2, lo:hi]

            def layer_norm(x4, g_ap, ln8_out, out_extra):
                """x4: 4 f32 [P,TOK] tiles at 64x. Writes normalized*g (no
                +bias: folded downstream) as fp8 into ln8_out (wide); calls
                out_extra(k, u32, eng) for extra outputs (u32 f32)."""
                m8 = sba.tile([P, KT * TOK], f8, tag="ln_m8", name="ln_m8")
                sq8 = sba.tile([P, KT * TOK], f8, tag="ln_sq8", name="ln_sq8")
                for k in range(KT):
                    cs = slice(TOK * k, TOK * (k + 1))
                    eng = (nc.gpsimd, nc.vector)[k % 2]
                    eng.tensor_scalar_mul(m8[:, cs], x4[k][:], 1.0 / WS)
                    eng.tensor_mul(sq8[:, cs], m8[:, cs], m8[:, cs])
                psum_s = psA.tile([P, TOK], f32, tag="ps", name="ps")
                psum_q = psA.tile([P, TOK], f32, tag="ps", name="ps")
                for k in (0, 2):
                    nc.tensor.matmul(psum_s[:], on2,
                                     pair2(m8[:], k, TOK, 0, TOK),
                                     start=(k == 0), stop=(k == 2),
                                     perf_mode=DR)
                    nc.tensor.matmul(psum_q[:], on2,
                                     pair2(sq8[:], k, TOK, 0, TOK),
                                     start=(k == 0), stop=(k == 2),
                                     perf_mode=DR)
                a = sba.tile([P, TOK], f32, tag="ln_a", name="ln_a")
                nc.vector.tensor_scalar_mul(a[:], psum_s[:], WS / D)
                a2 = sba.tile([P, TOK], f32, tag="ln_a2", name="ln_a2")
                nc.gpsimd.tensor_mul(a2[:], a[:], a[:])
                pt = sba.tile([P, TOK], f32, tag="ln_pt", name="ln_pt")
                nc.vector.scalar_tensor_tensor(
                    pt[:], in0=a2[:], scalar=float(-D / (WS * WS)),
                    in1=psum_q[:], op0=OP.mult, op1=OP.add)
                # rstd*C = exp(-0.5*ln(pt) + ln(C)), C = sqrt(D)/WS
                lnv = sba.tile([P, TOK], f32, tag="ln_lnv", name="ln_lnv")
                nc.scalar.activation(lnv[:], pt[:], AF.Ln, bias=0.0, scale=1.0)
                rstd = sba.tile([P, TOK], f32, tag="ln_rstd", name="ln_rstd")
                nc.scalar.activation(rstd[:], lnv[:], AF.Exp,
                                     bias=lnC[:, 0:1], scale=-0.5)
                for k in range(KT):
                    ei = k % 2
                    eng = (nc.vector, nc.gpsimd)[ei]
                    t = sba.tile([P, TOK], f32, tag=f"ln_t{ei}",
                                 name=f"ln_t{ei}")
                    eng.tensor_sub(t[:], x4[k][:], a[:])

                    def stt_mm(out_ap, tin):
                        # (t * g) * rstd; Pool lacks scalar_tensor_tensor
                        if ei == 0:
                            eng.scalar_tensor_tensor(
                                out_ap, in0=tin, scalar=g_ap(k), in1=rstd[:],
                                op0=OP.mult, op1=OP.mult)
                        else:
                            w = sba.tile([P, TOK], f32, tag="ln_w1",
                                         name="ln_w1")
                            eng.tensor_scalar_mul(w[:], tin, g_ap(k))
                            eng.tensor_mul(out_ap, w[:], rstd[:])

                    if out_extra is None:
                        stt_mm(ln8_out[:, TOK * k:TOK * (k + 1)], t[:])
                    else:
                        u = sba.tile([P, TOK], f32, tag=f"ln_u{ei}",
                                     name=f"ln_u{ei}")
                        stt_mm(u[:], t[:])
                        eng.tensor_copy(ln8_out[:, TOK * k:TOK * (k + 1)],
                                        u[:])
                        out_extra(k, u, eng)

            for rep in range(reps):
              for l in range(nb):
                # ---- weights: one DMA per tensor (fp8) ----
                wq_t = sbw.tile([P, KT * D], f8, tag="wq", name="wq")
                wk_t = sbw.tile([P, KT * D], f8, tag="wk", name="wk")
                wv_t = sbw.tile([P, KT * D], f8, tag="wv", name="wv")
                fc_t = sbw.tile([P, KT * D], f8, tag="fcw", name="fcw")
                w1_t = sbw.tile([P, KT * DHID], f16, tag="w1", name="w1")
                w2_t = sbw.tile([P, 8 * D], f16, tag="w2", name="w2")
                for sb_t, src, width in ((wk_t, wk_in, D), (wv_t, wv_in, D),
                                         (wq_t, wq_in, D), (fc_t, fc_in, D),
                                         (w1_t, w1_in, DHID)):
                    nc.sync.dma_start(
                        sb_t[:].rearrange("p (k c) -> p k c", c=width),
                        src[l].rearrange("(k p) c -> p k c", p=P))
                nc.sync.dma_start(
                    w2_t[:].rearrange("p (k c) -> p k c", c=D),
                    w2_in[l].rearrange("(k p) c -> p k c", p=P))
                bt = sbb.tile([P, 44], f32, tag="bias", name="bias")
                nc.sync.dma_start(bt[:], bias_in[l, :, :])

                def bap(name, idx):
                    o = BOF[name] + idx
                    return bt[:, o:o + 1]

                # ---- k^T (feature-major, fp8 at 64x), AG_K ----
                kT8 = sba.tile([P, KT * TOK], f8, tag="kT8", name="kT8")
                for hp in range(4):
                    ps = psA.tile([P, TOK], f32, tag="ps", name="ps")
                    for k in (0, 2):
                        nc.tensor.matmul(
                            ps[:], pair2(wk_t[:], k, D, P * hp, P * (hp + 1)),
                            pair2(x8w[:], k, TOK, 0, TOK),
                            start=(k == 0), stop=(k == 2), perf_mode=DR)
                    nc.scalar.activation(
                        kT8[:, TOK * hp:TOK * (hp + 1)], ps[:], AF.Identity,
                        bias=bap("bk", hp), scale=1.0)
                cc_i = dram.tile([D, TOK + D], f8, tag="cc_i", name="cc_i")
                cc_o = dram.tile([4 * D, TOK + D], f8, tag="cc_o",
                                 name="cc_o")
                cc_ok = cc_o[:, 0:TOK]
                nc.sync.dma_start(
                    cc_i[:, 0:TOK].rearrange("(hp p) c -> p hp c", p=P),
                    kT8[:].rearrange("p (hp c) -> p hp c", c=TOK))

                # ---- v (token-major, aug fp8 at 64x), AG_V ----
                for t in range(4):
                    ps = psA.tile([P, D], f32, tag="ps", name="ps")
                    for k in (0, 2):
                        nc.tensor.matmul(
                            ps[:], pair2(x8w[:], k, TOK, P * t, P * (t + 1)),
                            pair2(wv_t[:], k, D, 0, D),
                            start=(k == 0), stop=(k == 2), perf_mode=DR)
                    nc.vector.tensor_copy(v8[:, D * t:D * (t + 1)], ps[:])
                nc.sync.dma_start(
                    cc_i[:, TOK:TOK + D].rearrange("(t p) c -> p t c", p=P),
                    v8[:].rearrange("p (t c) -> p t c", c=D))
                if sim1:
                    for r in range(4):
                        nc.sync.dma_start(cc_o[D * r:D * (r + 1), :],
                                          cc_i[:, :])
                else:
                    nc.gpsimd.collective_compute(
                        "AllGather", mybir.AluOpType.bypass,
                        replica_groups=GROUPS,
                        ins=[cc_i[:].opt()], outs=[cc_o[:].opt()])

                # ---- q^T fp8 (overlaps the collectives) ----
                qT8 = sba.tile([P, KT * TOK], f8, tag="qT8", name="qT8")
                for hp in range(4):
                    ps = psA.tile([P, TOK], f32, tag="ps", name="ps")
                    for k in (0, 2):
                        nc.tensor.matmul(
                            ps[:], pair2(wq_t[:], k, D, P * hp, P * (hp + 1)),
                            pair2(x8w[:], k, TOK, 0, TOK),
                            start=(k == 0), stop=(k == 2), perf_mode=DR)
                    nc.vector.tensor_scalar_add(
                        qT8[:, TOK * hp:TOK * (hp + 1)], ps[:], bap("bq", hp))

                # ---- gather-in loads ----
                for p in range(4):
                    nc.sync.dma_start(
                        ktf[p][:].rearrange("p (c w) -> p c w", w=TOK),
                        cc_ok.rearrange(
                            "(c p q) w -> c p q w", p=4, q=P)[:, p, :, :]
                        .rearrange("c q w -> q c w"))
                for cch in range(4):
                    for j in range(4):
                        jj = 4 * cch + j
                        src = cc_o[D * cch + P * j:D * cch + P * (j + 1),
                                   TOK:TOK + D]
                        sr = src.rearrange("p (pr w b) -> p pr w b",
                                           pr=4, w=2)
                        dst = va_all[:, VW * jj:VW * (jj + 1)].rearrange(
                            "p (pr w2 b) -> p pr w2 b", pr=4, w2=3)
                        for w in range(2):
                            nc.sync.dma_start(dst[:, :, 2 * w, :],
                                              sr[:, :, w, :])

                # ---- attention, head pairs ----
                oT8w = sba.tile([P, KT * TOK], f8, tag="oT8w", name="oT8w")
                va_r = va_all[:].rearrange("p (j c) -> p j c", c=VW)
                for hp in range(4):
                    qs = qT8[:, TOK * hp:TOK * (hp + 1)]
                    po_e = psA.tile([P, TOK], f32, tag="ps", name="ps")
                    po_o = psA.tile([P, TOK], f32, tag="ps", name="ps")
                    e_list = []
                    for g in range(8):
                        ps_e = psS.tile([P, 1024], f32, tag="ps_sc",
                                        name="ps_sc")
                        ps_o = psS.tile([P, 1024], f32, tag="ps_sc",
                                        name="ps_sc")
                        for c in range(2):
                            j = 2 * g + c
                            nc.tensor.matmul(
                                ps_e[:, TOK * c:TOK * (c + 1)],
                                ktf[hp][0:64, P * j:P * (j + 1)],
                                qs[0:64, :], start=True, stop=True)
                            nc.tensor.matmul(
                                ps_o[:, TOK * c:TOK * (c + 1)],
                                ktf[hp][64:128, P * j:P * (j + 1)],
                                qs[64:128, :], start=True, stop=True)
                        e_e = sbe.tile([P, 1024], u8, tag="e", name="e")
                        e_o = sbe.tile([P, 1024], u8, tag="e", name="e")
                        # interleave: Act every g; Pool/DVE alternate
                        nc.scalar.activation(e_e[:].bitcast(f8), ps_e[:],
                                             AF.Exp, bias=0.0, scale=SEXP)
                        if g == 0:
                            nc.scalar.activation(e_o[:].bitcast(f8), ps_o[:],
                                                 AF.Exp, bias=0.0, scale=SEXP)
                        else:
                            nc.vector.tensor_scalar(e_o[:], ps_o[:], FE_A,
                                                    FE_B, OP.mult, OP.add)
                        e_list.append((e_e, e_o))
                        if g >= 1:
                            pe_, po_ = e_list[g - 1]
                            jg = g - 1
                            nc.tensor.matmul(
                                po_e[:],
                                va_r[:, 2 * jg:2 * jg + 2,
                                     192 * hp:192 * hp + 128],
                                pe_[:].bitcast(f8).rearrange(
                                    "p (t c) -> p t c", t=2),
                                start=(jg == 0), stop=False, perf_mode=DR)
                            nc.tensor.matmul(
                                po_o[:],
                                va_r[:, 2 * jg:2 * jg + 2,
                                     192 * hp + 64:192 * hp + 192],
                                po_[:].bitcast(f8).rearrange(
                                    "p (t c) -> p t c", t=2),
                                start=(jg == 0), stop=False, perf_mode=DR)
                    pe_, po_ = e_list[7]
                    nc.tensor.matmul(
                        po_e[:], va_r[:, 14:16, 192 * hp:192 * hp + 128],
                        pe_[:].bitcast(f8).rearrange("p (t c) -> p t c", t=2),
                        start=False, stop=True, perf_mode=DR)
                    nc.tensor.matmul(
                        po_o[:], va_r[:, 14:16, 192 * hp + 64:192 * hp + 192],
                        po_[:].bitcast(f8).rearrange("p (t c) -> p t c", t=2),
                        start=False, stop=True, perf_mode=DR)
                    # normalize: denominators are rows dl; o rows ol
                    for par, po in ((0, po_e), (1, po_o)):
                        ol = slice(64 * par, 64 * par + 64)
                        dl = slice(64 * (1 - par), 64 * (1 - par) + 64)
                        dcp = sba.tile([P, TOK], f16, tag=f"dcp{par}",
                                       name=f"dcp{par}")
                        nc.scalar.activation(dcp[dl, :], po[dl, :], AF.Copy,
                                             bias=0.0, scale=1.0 / 64.0)
                        ps2 = psA.tile([P, TOK], f32, tag="ps", name="ps")
                        nc.tensor.matmul(ps2[:], ones16[dl, :], dcp[dl, :],
                                         start=True, stop=True)
                        rec = sba.tile([P, TOK], f32, tag=f"rec{par}",
                                       name=f"rec{par}")
                        nc.vector.reciprocal_approx_fast(rec[:], ps2[:])
                        nc.vector.scalar_tensor_tensor(
                            oT8w[:, TOK * hp:TOK * (hp + 1)][ol, :],
                            in0=po[ol, :], scalar=float(1.0 / WS),
                            in1=rec[ol, :], op0=OP.mult, op1=OP.mult)

                # ---- fc + residual (xmid = 64x) ----
                xmid = []
                for m in range(4):
                    ps = psA.tile([P, TOK], f32, tag="ps", name="ps")
                    for k in (0, 2):
                        nc.tensor.matmul(
                            ps[:], pair2(fc_t[:], k, D, P * m, P * (m + 1)),
                            pair2(oT8w[:], k, TOK, 0, TOK),
                            start=(k == 0), stop=(k == 2), perf_mode=DR)
                    xm = sba.tile([P, TOK], f32, tag=f"xmid{m}",
                                  name=f"xmid{m}")
                    nc.vector.scalar_tensor_tensor(
                        xm[:], in0=ps[:], scalar=bap("fcb", m), in1=xin[m][:],
                        op0=OP.add, op1=OP.add)
                    xmid.append(xm)

                # ---- LN1 -> MLP ----
                ln8w = sba.tile([P, KT * TOK], f16, tag="ln8w", name="ln8w")
                layer_norm(xmid, lambda k: bap("g1", k), ln8w, None)
                h8w = sba.tile([P, 8 * TOK], f16, tag="h8w", name="h8w")
                for m in range(8):
                    ps = psA.tile([P, TOK], f32, tag="ps", name="ps")
                    for k in range(KT):
                        nc.tensor.matmul(
                            ps[:],
                            w1_t[:, DHID * k + P * m:DHID * k + P * (m + 1)],
                            ln8w[:, TOK * k:TOK * (k + 1)],
                            start=(k == 0), stop=(k == KT - 1))
                    nc.scalar.activation(h8w[:, TOK * m:TOK * (m + 1)], ps[:],
                                         AF.Relu, bias=bap("b1", m), scale=1.0)
                xout = []
                for m in range(4):
                    ps = psA.tile([P, TOK], f32, tag="ps", name="ps")
                    for k in range(8):
                        nc.tensor.matmul(
                            ps[:], w2_t[:, D * k + P * m:D * k + P * (m + 1)],
                            h8w[:, TOK * k:TOK * (k + 1)],
                            start=(k == 0), stop=(k == 7))
                    tb = sba.tile([P, TOK], f32, tag=f"tb{m}", name=f"tb{m}")
                    nc.vector.tensor_scalar(tb[:], ps[:], 1.0 / WS,
                                            bap("b2", m), OP.mult, OP.add)
                    xo = sba.tile([P, TOK], f32, tag=f"xout{m}",
                                  name=f"xout{m}")
                    nc.gpsimd.tensor_add(xo[:], tb[:], xmid[m][:])
                    xout.append(xo)

                # ---- LN2 -> next block's x (fp8 sans bias + f32@64x) ----
                x8w = sba.tile([P, KT * TOK], f8, tag="x8w", name="x8w")
                xin = [sba.tile([P, TOK], f32, tag=f"x32_{k}",
                                name=f"x32_{k}") for k in range(KT)]
                last = (rep == reps - 1) and (l == nb - 1)

                if last:
                    def extra(k, u, eng, xin=xin):
                        eng.tensor_scalar_add(xin[k][:], u[:], bap("be2", k))
                else:
                    def extra(k, u, eng, xin=xin):
                        eng.tensor_scalar(xin[k][:], u[:], WS,
                                          bap("be2s", k), OP.mult, OP.add)
                layer_norm(xout, lambda k: bap("g2", k), x8w, extra)

            for k in range(KT):
                nc.sync.dma_start(yT_out[P * k:P * (k + 1), :], xin[k][:])

    bacc_mod.get_activation_tables = _patched
    hw_specs.get_activation_tables = _patched
    try:
        nc.compile()
    finally:
        bacc_mod.get_activation_tables = _orig_tables
        hw_specs.get_activation_tables = _orig_tables
    return nc


def _host_prep(inputs, nb):
    import ml_dtypes
    f8t = ml_dtypes.float8_e4m3fn
    qkv_w = np.asarray(inputs["qkv_w"], dtype=np.float32)[:nb]
    qkv_b = np.asarray(inputs["qkv_b"], dtype=np.float32)[:nb]
    fc_w = np.asarray(inputs["fc_w"], dtype=np.float32)[:nb]
    fc_b = np.asarray(inputs["fc_b"], dtype=np.float32)[:nb]
    w1 = np.asarray(inputs["w1"], dtype=np.float32)[:nb]
    b1 = np.asarray(inputs["b1"], dtype=np.float32)[:nb]
    w2 = np.asarray(inputs["w2"], dtype=np.float32)[:nb]
    b2 = np.asarray(inputs["b2"], dtype=np.float32)[:nb]
    g1 = np.asarray(inputs["ln1_g"], dtype=np.float32)[:nb]
    be1 = np.asarray(inputs["ln1_b"], dtype=np.float32)[:nb]
    g2 = np.asarray(inputs["ln2_g"], dtype=np.float32)[:nb]
    be2 = np.asarray(inputs["ln2_b"], dtype=np.float32)[:nb]

    idx_q = np.concatenate([np.arange(192 * h, 192 * h + 64)
                            for h in range(H)])
    idx_k = idx_q + 64
    idx_v = idx_q + 128

    def btile(b, nt):  # [nb, N] -> [nb, P, nt] with [l, p, m] = b[l, 128m+p]
        return b.reshape(nb, nt, P).transpose(0, 2, 1)

    wq_s = qkv_w[:, :, idx_q]
    wk_s = qkv_w[:, :, idx_k]
    wv_s = qkv_w[:, :, idx_v]
    # LN output biases are not applied on-device; fold them into the next
    # layer's biases. Block l>0 inputs x8 lack be2[l-1]; LN1 output lacks be1.
    dt64 = np.float64
    prev_be2 = np.concatenate(
        [np.zeros((1, D), np.float32), be2[:-1]], axis=0).astype(dt64)
    bq_eff = qkv_b[:, idx_q] + np.einsum(
        "ld,ldf->lf", prev_be2, wq_s.astype(dt64)).astype(np.float32)
    bk_eff = qkv_b[:, idx_k] + np.einsum(
        "ld,ldf->lf", prev_be2, wk_s.astype(dt64)).astype(np.float32)
    bv_eff = qkv_b[:, idx_v] + np.einsum(
        "ld,ldf->lf", prev_be2, wv_s.astype(dt64)).astype(np.float32)
    fcb_eff = fc_b + np.einsum("ld,ldf->lf", bv_eff.astype(dt64),
                               fc_w.astype(dt64)).astype(np.float32)
    b1_eff = b1 + np.einsum("ld,ldf->lf", be1.astype(dt64),
                            w1.astype(dt64)).astype(np.float32)
    biases = np.concatenate([
        btile(bq_eff * WS, 4), btile(bk_eff * WS, 4),
        btile(fcb_eff * WS, 4), btile(b2 * WS, 4), btile(g1, 4),
        btile(be1, 4), btile(g2, 4), btile(be2, 4), btile(b1_eff * WS, 8),
        btile(be2 * WS, 4)], axis=2)

    def w8(a):
        return np.ascontiguousarray(a * WS).astype(f8t)

    common = {
        "wq": w8(wq_s),
        "wk": w8(wk_s),
        "wv": w8(wv_s),
        "fcw": w8(fc_w),
        "w1": np.ascontiguousarray(w1 * WS).astype(np.float16),
        "w2": np.ascontiguousarray(w2 * WS).astype(np.float16),
        "biases": np.ascontiguousarray(biases),
    }
    X = np.asarray(inputs["X"], dtype=np.float32)
    in_maps = []
    for c in range(N_CORES):
        b, r = c // 4, c % 4
        xT = np.ascontiguousarray(X[b, TOK * r:TOK * (r + 1), :].T) * WS
        in_maps.append({"xT": xT, **common})
    return in_maps


def get_nc(nb=NB, reps=1):
    key = (nb, reps)
    if key not in _CACHE:
        _CACHE[key] = _build(nb, reps)
    return _CACHE[key]


def kernel(**inputs):
    from concourse.bass_utils import run_bass_kernel_spmd

    nb = NB
    nc = get_nc(nb)
    in_maps = _host_prep(inputs, nb)
    res = run_bass_kernel_spmd(nc, in_maps, list(range(N_CORES)))
    Y = np.zeros((B, S, D), dtype=np.float32)
    for c in range(N_CORES):
        b, r = c // 4, c % 4
        Y[b, TOK * r:TOK * (r + 1), :] = res.results[c]["yT"].T
    return Y


# revision 14
# speedup vs baseline: 1.2064x; 1.1347x over previous
"""12-block transformer encoder (B=2, S=2048, D=512, H=8, DHID=1024) on 8 trn2 cores.

Sequence-parallel: core c owns batch c//4, tokens 512*(c%4)..+512. Weights
replicated, fp8e4m3 at 64x scale (escapes e4m3 denormals; folds undone in
existing scalar slots). Residual stream f32 at 64x. Projections + attn@V run
fp8 DoubleRow (2 contraction chunks per matmul, 0.5 cyc/row). Scores fp8
1 cyc/row. Softmax exp split across engines: exact Exp on Activation, bit-trick
fast-exp (tensor_scalar -> uint8 bits viewed as fp8e4) on Pool and DVE,
interleaved per score-group so all three engines run concurrently.
MLP (w1/w2) stays fp16: fp8 quantization noise there does not average down
and was the dominant error term. LayerNorm rstd = Exp(-0.5*Ln(D*var)+lnC) on
Activation; a get_activation_tables patch during build makes the table pass
pick the single set containing exp+ln+relu, so there is exactly one table
load. LN output biases are folded into the next layer's biases host-side.
K^T and plain V AllGathered in fp8 as one collective; the gather-in DMAs
spread V into the aug layout (persistent ones columns in SBUF supply the
softmax denominator through the attn@V DoubleRow matmul).
"""
import sys
import numpy as np

for _p in ("/opt/trn_rl_repo", "/root/.axon_site/_ro/trn_rl_repo"):
    if _p not in sys.path:
        sys.path.insert(0, _p)

P = 128
B, S, D = 2, 2048, 512
H, DH, DHID = 8, 64, 1024
NB = 12
TOK = 512            # tokens per core
KT = D // P          # 4 contraction chunks over D
N_CORES = 8
GROUPS = [[0, 1, 2, 3], [4, 5, 6, 7]]
VW = 768             # aug-v row width per token ptile (4 pairs x 192)
WS = 64.0            # weight scale
LOG2E = 1.4426950408889634
SPLIT_AG = False

_CACHE = {}


def _build(nb, reps=1):
    import os
    import concourse.bass as bass
    import concourse.mybir as mybir
    import concourse.tile as tile
    from concourse import bacc
    import concourse.hw_specs as hw_specs
    import concourse.bacc as bacc_mod

    f32 = mybir.dt.float32
    f16 = mybir.dt.float16
    f8 = mybir.dt.float8e4
    u8 = mybir.dt.uint8
    AF = mybir.ActivationFunctionType
    OP = mybir.AluOpType
    DR = mybir.MatmulPerfMode.DoubleRow

    # Steer the act-table pass to the one set holding exp+ln+relu so the
    # kernel needs a single table load. Runtime-correct: we only narrow the
    # claimed contents of the other sets; ids stay the json indices.
    _orig_tables = hw_specs.get_activation_tables

    def _patched(arch):
        t = _orig_tables(arch)
        for name, s in t.items():
            if name != "natural_log_exp_and_others":
                s.discard(AF.Exp)
                s.discard(AF.Ln)
                s.discard(AF.Relu)
        return t

    sim1 = os.environ.get("KSIM") in ("1", "2")
    sim_cheap = os.environ.get("KSIM") == "2"
    nc = bacc.Bacc("TRN2", target_bir_lowering=False, debug=False,
                   num_devices=(1 if sim1 else N_CORES))

    xT_in = nc.declare_dram_parameter("xT", [D, TOK], f32, isOutput=False)
    wq_in = nc.declare_dram_parameter("wq", [nb, D, D], f8, isOutput=False)
    wk_in = nc.declare_dram_parameter("wk", [nb, D, D], f8, isOutput=False)
    wv_in = nc.declare_dram_parameter("wv", [nb, D, D], f8, isOutput=False)
    fc_in = nc.declare_dram_parameter("fcw", [nb, D, D], f8, isOutput=False)
    w1_in = nc.declare_dram_parameter("w1", [nb, D, DHID], f16, isOutput=False)
    w2_in = nc.declare_dram_parameter("w2", [nb, DHID, D], f16, isOutput=False)
    bias_in = nc.declare_dram_parameter("biases", [nb, P, 44], f32,
                                        isOutput=False)
    yT_out = nc.declare_dram_parameter("yT", [D, TOK], f32, isOutput=True)

    inv_sqrt_d = float(1.0 / np.sqrt(D))
    SEXP = inv_sqrt_d / (WS * WS)     # exp arg = psum * SEXP
    FE_A = LOG2E * 8.0 * SEXP         # fast-exp bits = psum*FE_A + FE_B
    FE_B = 7.0 * 8.0 - 0.58
    BOF = {"bq": 0, "bk": 4, "fcb": 8, "b2": 12, "g1": 16, "be1": 20,
           "g2": 24, "be2": 28, "b1": 32, "be2s": 40}

    with tile.TileContext(nc) as tc:
        with tc.tile_pool(name="w", bufs=3) as sbw, \
             tc.tile_pool(name="bias", bufs=3) as sbb, \
             tc.tile_pool(name="act", bufs=1) as sba, \
             tc.tile_pool(name="e", bufs=12) as sbe, \
             tc.tile_pool(name="pers", bufs=1) as sbp, \
             tc.tile_pool(name="psS", bufs=2, space="PSUM") as psS, \
             tc.tile_pool(name="psA", bufs=4, space="PSUM") as psA, \
             tc.tile_pool(name="dram", bufs=2, space="DRAM") as dram:

            ones16 = sbp.tile([P, P], f16, tag="ones16", name="ones16")
            nc.vector.memset(ones16[:], 1.0)
            lnC = sbp.tile([P, 1], f32, tag="lnC", name="lnC")
            nc.vector.memset(lnC[:], float(np.log(np.sqrt(D) / WS)))
            ones8 = sbp.tile([P, 2 * P], f8, tag="ones8", name="ones8")
            nc.vector.memset(ones8[:], 1.0)
            on2 = ones8[:].rearrange("p (t m) -> p t m", t=2)
            ktf = [sbp.tile([P, S], f8, tag=f"ktf{p}", name=f"ktf{p}")
                   for p in range(4)]
            # all 16 key-ptile aug-v tiles in one tensor: [P, 16*768] fp8
            va_all = sbp.tile([P, 16 * VW], f8, tag="va", name="va")
            # own-token plain v staging (head-major); aug ones only in va_all
            v8 = sbp.tile([P, KT * D], f8, tag="v8", name="v8")
            oc = va_all[:].rearrange("p (a b) -> p a b", b=64)
            nc.vector.memset(oc[:, 1:oc.shape[1]:3, :], 1.0)

            # residual stream (64x) + fp8 copy
            xin = [sba.tile([P, TOK], f32, tag=f"x32_{k}", name=f"x32_{k}")
                   for k in range(KT)]
            x8w = sba.tile([P, KT * TOK], f8, tag="x8w", name="x8w")
            for k in range(KT):
                nc.sync.dma_start(xin[k][:], xT_in[P * k:P * (k + 1), :])
                nc.vector.tensor_scalar_mul(
                    x8w[:, TOK * k:TOK * (k + 1)], xin[k][:], 1.0 / WS)

            def pair2(ap_wide, k, width, lo, hi):
                """[P, 2, hi-lo] view: chunks k,k+1 of ap_wide, cols lo:hi."""
                r = ap_wide.rearrange("p (k c) -> p k c", c=width)
                return r[:, k:k + 2, lo:hi]

            def layer_norm(x4, g_ap, ln8_out, out_extra):
                """x4: 4 f32 [P,TOK] tiles at 64x. Writes normalized*g (no
                +bias: folded downstream) as fp8 into ln8_out (wide); calls
                out_extra(k, u32, eng) for extra outputs (u32 f32)."""
                m8 = sba.tile([P, KT * TOK], f8, tag="ln_m8", name="ln_m8")
                sq8 = sba.tile([P, KT * TOK], f8, tag="ln_sq8", name="ln_sq8")
                for k in range(KT):
                    cs = slice(TOK * k, TOK * (k + 1))
                    eng = (nc.gpsimd, nc.vector)[k % 2]
                    eng.tensor_scalar_mul(m8[:, cs], x4[k][:], 1.0 / WS)
                    eng.tensor_mul(sq8[:, cs], m8[:, cs], m8[:, cs])
                psum_s = psA.tile([P, TOK], f32, tag="ps", name="ps")
                psum_q = psA.tile([P, TOK], f32, tag="ps", name="ps")
                for k in (0, 2):
                    nc.tensor.matmul(psum_s[:], on2,
                                     pair2(m8[:], k, TOK, 0, TOK),
                                     start=(k == 0), stop=(k == 2),
                                     perf_mode=DR)
                    nc.tensor.matmul(psum_q[:], on2,
                                     pair2(sq8[:], k, TOK, 0, TOK),
                                     start=(k == 0), stop=(k == 2),
                                     perf_mode=DR)
                a = sba.tile([P, TOK], f32, tag="ln_a", name="ln_a")
                nc.vector.tensor_scalar_mul(a[:], psum_s[:], WS / D)
                a2 = sba.tile([P, TOK], f32, tag="ln_a2", name="ln_a2")
                nc.gpsimd.tensor_mul(a2[:], a[:], a[:])
                pt = sba.tile([P, TOK], f32, tag="ln_pt", name="ln_pt")
                nc.vector.scalar_tensor_tensor(
                    pt[:], in0=a2[:], scalar=float(-D / (WS * WS)),
                    in1=psum_q[:], op0=OP.mult, op1=OP.add)
                # rstd*C = exp(-0.5*ln(pt) + ln(C)), C = sqrt(D)/WS
                lnv = sba.tile([P, TOK], f32, tag="ln_lnv", name="ln_lnv")
                nc.scalar.activation(lnv[:], pt[:], AF.Ln, bias=0.0, scale=1.0)
                rstd = sba.tile([P, TOK], f32, tag="ln_rstd", name="ln_rstd")
                nc.scalar.activation(rstd[:], lnv[:], AF.Exp,
                                     bias=lnC[:, 0:1], scale=-0.5)
                for k in range(KT):
                    ei = k % 2
                    eng = (nc.vector, nc.gpsimd)[ei]
                    t = sba.tile([P, TOK], f32, tag=f"ln_t{ei}",
                                 name=f"ln_t{ei}")
                    eng.tensor_sub(t[:], x4[k][:], a[:])

                    def stt_mm(out_ap, tin):
                        # (t * g) * rstd; Pool lacks scalar_tensor_tensor
                        if ei == 0:
                            eng.scalar_tensor_tensor(
                                out_ap, in0=tin, scalar=g_ap(k), in1=rstd[:],
                                op0=OP.mult, op1=OP.mult)
                        else:
                            w = sba.tile([P, TOK], f32, tag="ln_w1",
                                         name="ln_w1")
                            eng.tensor_scalar_mul(w[:], tin, g_ap(k))
                            eng.tensor_mul(out_ap, w[:], rstd[:])

                    if out_extra is None:
                        stt_mm(ln8_out[:, TOK * k:TOK * (k + 1)], t[:])
                    else:
                        u = sba.tile([P, TOK], f32, tag=f"ln_u{ei}",
                                     name=f"ln_u{ei}")
                        stt_mm(u[:], t[:])
                        eng.tensor_copy(ln8_out[:, TOK * k:TOK * (k + 1)],
                                        u[:])
                        out_extra(k, u, eng)

            for rep in range(reps):
              for l in range(nb):
                # ---- weights: one DMA per tensor (fp8) ----
                wq_t = sbw.tile([P, KT * D], f8, tag="wq", name="wq")
                wk_t = sbw.tile([P, KT * D], f8, tag="wk", name="wk")
                wv_t = sbw.tile([P, KT * D], f8, tag="wv", name="wv")
                fc_t = sbw.tile([P, KT * D], f8, tag="fcw", name="fcw")
                w1_t = sbw.tile([P, KT * DHID], f16, tag="w1", name="w1")
                w2_t = sbw.tile([P, 8 * D], f16, tag="w2", name="w2")
                for sb_t, src, width in ((wk_t, wk_in, D), (wv_t, wv_in, D),
                                         (wq_t, wq_in, D), (fc_t, fc_in, D),
                                         (w1_t, w1_in, DHID)):
                    nc.sync.dma_start(
                        sb_t[:].rearrange("p (k c) -> p k c", c=width),
                        src[l].rearrange("(k p) c -> p k c", p=P))
                nc.sync.dma_start(
                    w2_t[:].rearrange("p (k c) -> p k c", c=D),
                    w2_in[l].rearrange("(k p) c -> p k c", p=P))
                bt = sbb.tile([P, 44], f32, tag="bias", name="bias")
                nc.sync.dma_start(bt[:], bias_in[l, :, :])

                def bap(name, idx):
                    o = BOF[name] + idx
                    return bt[:, o:o + 1]

                # ---- k^T (feature-major, fp8 at 64x), AG_K ----
                kT8 = sba.tile([P, KT * TOK], f8, tag="kT8", name="kT8")
                for hp in range(4):
                    ps = psA.tile([P, TOK], f32, tag="ps", name="ps")
                    for k in (0, 2):
                        nc.tensor.matmul(
                            ps[:], pair2(wk_t[:], k, D, P * hp, P * (hp + 1)),
                            pair2(x8w[:], k, TOK, 0, TOK),
                            start=(k == 0), stop=(k == 2), perf_mode=DR)
                    nc.scalar.activation(
                        kT8[:, TOK * hp:TOK * (hp + 1)], ps[:], AF.Identity,
                        bias=bap("bk", hp), scale=1.0)
                cc_i = dram.tile([D, TOK + D], f8, tag="cc_i", name="cc_i")
                cc_o = dram.tile([4 * D, TOK + D], f8, tag="cc_o",
                                 name="cc_o")
                cc_ok = cc_o[:, 0:TOK]
                nc.sync.dma_start(
                    cc_i[:, 0:TOK].rearrange("(hp p) c -> p hp c", p=P),
                    kT8[:].rearrange("p (hp c) -> p hp c", c=TOK))

                # ---- v (token-major, aug fp8 at 64x), AG_V ----
                for t in range(4):
                    ps = psA.tile([P, D], f32, tag="ps", name="ps")
                    for k in (0, 2):
                        nc.tensor.matmul(
                            ps[:], pair2(x8w[:], k, TOK, P * t, P * (t + 1)),
                            pair2(wv_t[:], k, D, 0, D),
                            start=(k == 0), stop=(k == 2), perf_mode=DR)
                    nc.vector.tensor_copy(v8[:, D * t:D * (t + 1)], ps[:])
                nc.sync.dma_start(
                    cc_i[:, TOK:TOK + D].rearrange("(t p) c -> p t c", p=P),
                    v8[:].rearrange("p (t c) -> p t c", c=D))
                if sim1:
                    for r in range(4):
                        nc.sync.dma_start(cc_o[D * r:D * (r + 1), :],
                                          cc_i[:, :])
                else:
                    nc.gpsimd.collective_compute(
                        "AllGather", mybir.AluOpType.bypass,
                        replica_groups=GROUPS,
                        ins=[cc_i[:].opt()], outs=[cc_o[:].opt()])

                # ---- q^T fp8 (overlaps the collectives) ----
                qT8 = sba.tile([P, KT * TOK], f8, tag="qT8", name="qT8")
                for hp in range(4):
                    ps = psA.tile([P, TOK], f32, tag="ps", name="ps")
                    for k in (0, 2):
                        nc.tensor.matmul(
                            ps[:], pair2(wq_t[:], k, D, P * hp, P * (hp + 1)),
                            pair2(x8w[:], k, TOK, 0, TOK),
                            start=(k == 0), stop=(k == 2), perf_mode=DR)
                    nc.vector.tensor_scalar_add(
                        qT8[:, TOK * hp:TOK * (hp + 1)], ps[:], bap("bq", hp))

                # ---- gather-in loads ----
                for p in range(4):
                    nc.sync.dma_start(
                        ktf[p][:].rearrange("p (c w) -> p c w", w=TOK),
                        cc_ok.rearrange(
                            "(c p q) w -> c p q w", p=4, q=P)[:, p, :, :]
                        .rearrange("c q w -> q c w"))
                for cch in range(4):
                    for j in range(4):
                        jj = 4 * cch + j
                        src = cc_o[D * cch + P * j:D * cch + P * (j + 1),
                                   TOK:TOK + D]
                        sr = src.rearrange("p (pr w b) -> p pr w b",
                                           pr=4, w=2)
                        dst = va_all[:, VW * jj:VW * (jj + 1)].rearrange(
                            "p (pr w2 b) -> p pr w2 b", pr=4, w2=3)
                        for w in range(2):
                            nc.sync.dma_start(dst[:, :, 2 * w, :],
                                              sr[:, :, w, :])

                # ---- attention, head pairs ----
                oT8w = sba.tile([P, KT * TOK], f8, tag="oT8w", name="oT8w")
                va_r = va_all[:].rearrange("p (j c) -> p j c", c=VW)
                for hp in range(4):
                    qs = qT8[:, TOK * hp:TOK * (hp + 1)]
                    po_e = psA.tile([P, TOK], f32, tag="ps", name="ps")
                    po_o = psA.tile([P, TOK], f32, tag="ps", name="ps")
                    e_list = []
                    for g in range(8):
                        ps_e = psS.tile([P, 1024], f32, tag="ps_sc",
                                        name="ps_sc")
                        ps_o = psS.tile([P, 1024], f32, tag="ps_sc",
                                        name="ps_sc")
                        for c in range(2):
                            j = 2 * g + c
                            nc.tensor.matmul(
                                ps_e[:, TOK * c:TOK * (c + 1)],
                                ktf[hp][0:64, P * j:P * (j + 1)],
                                qs[0:64, :], start=True, stop=True)
                            nc.tensor.matmul(
                                ps_o[:, TOK * c:TOK * (c + 1)],
                                ktf[hp][64:128, P * j:P * (j + 1)],
                                qs[64:128, :], start=True, stop=True)
                        e_e = sbe.tile([P, 1024], u8, tag="e", name="e")
                        e_o = sbe.tile([P, 1024], u8, tag="e", name="e")
                        # interleave: Act every g; Pool/DVE alternate
                        nc.scalar.activation(e_e[:].bitcast(f8), ps_e[:],
                                             AF.Exp, bias=0.0, scale=SEXP)
                        if g == 0:
                            nc.scalar.activation(e_o[:].bitcast(f8), ps_o[:],
                                                 AF.Exp, bias=0.0, scale=SEXP)
                        else:
                            nc.vector.tensor_scalar(e_o[:], ps_o[:], FE_A,
                                                    FE_B, OP.mult, OP.add)
                        e_list.append((e_e, e_o))
                        if g >= 1:
                            pe_, po_ = e_list[g - 1]
                            jg = g - 1
                            nc.tensor.matmul(
                                po_e[:],
                                va_r[:, 2 * jg:2 * jg + 2,
                                     192 * hp:192 * hp + 128],
                                pe_[:].bitcast(f8).rearrange(
                                    "p (t c) -> p t c", t=2),
                                start=(jg == 0), stop=False, perf_mode=DR)
                            nc.tensor.matmul(
                                po_o[:],
                                va_r[:, 2 * jg:2 * jg + 2,
                                     192 * hp + 64:192 * hp + 192],
                                po_[:].bitcast(f8).rearrange(
                                    "p (t c) -> p t c", t=2),
                                start=(jg == 0), stop=False, perf_mode=DR)
                    pe_, po_ = e_list[7]
                    nc.tensor.matmul(
                        po_e[:], va_r[:, 14:16, 192 * hp:192 * hp + 128],
                        pe_[:].bitcast(f8).rearrange("p (t c) -> p t c", t=2),
                        start=False, stop=True, perf_mode=DR)
                    nc.tensor.matmul(
                        po_o[:], va_r[:, 14:16, 192 * hp + 64:192 * hp + 192],
                        po_[:].bitcast(f8).rearrange("p (t c) -> p t c", t=2),
                        start=False, stop=True, perf_mode=DR)
                    # normalize: denominators are rows dl; o rows ol
                    for par, po in ((0, po_e), (1, po_o)):
                        ol = slice(64 * par, 64 * par + 64)
                        dl = slice(64 * (1 - par), 64 * (1 - par) + 64)
                        dcp = sba.tile([P, TOK], f16, tag=f"dcp{par}",
                                       name=f"dcp{par}")
                        nc.scalar.activation(dcp[dl, :], po[dl, :], AF.Copy,
                                             bias=0.0, scale=1.0 / 64.0)
                        ps2 = psA.tile([P, TOK], f32, tag="ps", name="ps")
                        nc.tensor.matmul(ps2[:], ones16[dl, :], dcp[dl, :],
                                         start=True, stop=True)
                        rec = sba.tile([P, TOK], f32, tag=f"rec{par}",
                                       name=f"rec{par}")
                        nc.vector.reciprocal_approx_fast(rec[:], ps2[:])
                        nc.vector.scalar_tensor_tensor(
                            oT8w[:, TOK * hp:TOK * (hp + 1)][ol, :],
                            in0=po[ol, :], scalar=float(1.0 / WS),
                            in1=rec[ol, :], op0=OP.mult, op1=OP.mult)

                # ---- fc + residual (xmid = 64x) ----
                xmid = []
                for m in range(4):
                    ps = psA.tile([P, TOK], f32, tag="ps", name="ps")
                    for k in (0, 2):
                        nc.tensor.matmul(
                            ps[:], pair2(fc_t[:], k, D, P * m, P * (m + 1)),
                            pair2(oT8w[:], k, TOK, 0, TOK),
                            start=(k == 0), stop=(k == 2), perf_mode=DR)
                    xm = sba.tile([P, TOK], f32, tag=f"xmid{m}",
                                  name=f"xmid{m}")
                    nc.vector.scalar_tensor_tensor(
                        xm[:], in0=ps[:], scalar=bap("fcb", m), in1=xin[m][:],
                        op0=OP.add, op1=OP.add)
                    xmid.append(xm)

                # ---- LN1 -> MLP ----
                ln8w = sba.tile([P, KT * TOK], f16, tag="ln8w", name="ln8w")
                layer_norm(xmid, lambda k: bap("g1", k), ln8w, None)
                h8w = sba.tile([P, 8 * TOK], f16, tag="h8w", name="h8w")
                for m in range(8):
                    ps = psA.tile([P, TOK], f32, tag="ps", name="ps")
                    for k in range(KT):
                        nc.tensor.matmul(
                            ps[:],
                            w1_t[:, DHID * k + P * m:DHID * k + P * (m + 1)],
                            ln8w[:, TOK * k:TOK * (k + 1)],
                            start=(k == 0), stop=(k == KT - 1))
                    nc.scalar.activation(h8w[:, TOK * m:TOK * (m + 1)], ps[:],
                                         AF.Relu, bias=bap("b1", m), scale=1.0)
                xout = []
                for m in range(4):
                    ps = psA.tile([P, TOK], f32, tag="ps", name="ps")
                    for k in range(8):
                        nc.tensor.matmul(
                            ps[:], w2_t[:, D * k + P * m:D * k + P * (m + 1)],
                            h8w[:, TOK * k:TOK * (k + 1)],
                            start=(k == 0), stop=(k == 7))
                    tb = sba.tile([P, TOK], f32, tag=f"tb{m}", name=f"tb{m}")
                    nc.vector.tensor_scalar(tb[:], ps[:], 1.0 / WS,
                                            bap("b2", m), OP.mult, OP.add)
                    xo = sba.tile([P, TOK], f32, tag=f"xout{m}",
                                  name=f"xout{m}")
                    nc.gpsimd.tensor_add(xo[:], tb[:], xmid[m][:])
                    xout.append(xo)

                # ---- LN2 -> next block's x (fp8 sans bias + f32@64x) ----
                x8w = sba.tile([P, KT * TOK], f8, tag="x8w", name="x8w")
                xin = [sba.tile([P, TOK], f32, tag=f"x32_{k}",
                                name=f"x32_{k}") for k in range(KT)]
                last = (rep == reps - 1) and (l == nb - 1)

                if last:
                    def extra(k, u, eng, xin=xin):
                        eng.tensor_scalar_add(xin[k][:], u[:], bap("be2", k))
                else:
                    def extra(k, u, eng, xin=xin):
                        eng.tensor_scalar(xin[k][:], u[:], WS,
                                          bap("be2s", k), OP.mult, OP.add)
                layer_norm(xout, lambda k: bap("g2", k), x8w, extra)

            for k in range(KT):
                nc.sync.dma_start(yT_out[P * k:P * (k + 1), :], xin[k][:])

    bacc_mod.get_activation_tables = _patched
    hw_specs.get_activation_tables = _patched
    try:
        nc.compile()
    finally:
        bacc_mod.get_activation_tables = _orig_tables
        hw_specs.get_activation_tables = _orig_tables
    return nc


def _host_prep(inputs, nb):
    import ml_dtypes
    f8t = ml_dtypes.float8_e4m3fn
    qkv_w = np.asarray(inputs["qkv_w"], dtype=np.float32)[:nb]
    qkv_b = np.asarray(inputs["qkv_b"], dtype=np.float32)[:nb]
    fc_w = np.asarray(inputs["fc_w"], dtype=np.float32)[:nb]
    fc_b = np.asarray(inputs["fc_b"], dtype=np.float32)[:nb]
    w1 = np.asarray(inputs["w1"], dtype=np.float32)[:nb]
    b1 = np.asarray(inputs["b1"], dtype=np.float32)[:nb]
    w2 = np.asarray(inputs["w2"], dtype=np.float32)[:nb]
    b2 = np.asarray(inputs["b2"], dtype=np.float32)[:nb]
    g1 = np.asarray(inputs["ln1_g"], dtype=np.float32)[:nb]
    be1 = np.asarray(inputs["ln1_b"], dtype=np.float32)[:nb]
    g2 = np.asarray(inputs["ln2_g"], dtype=np.float32)[:nb]
    be2 = np.asarray(inputs["ln2_b"], dtype=np.float32)[:nb]

    idx_q = np.concatenate([np.arange(192 * h, 192 * h + 64)
                            for h in range(H)])
    idx_k = idx_q + 64
    idx_v = idx_q + 128

    def btile(b, nt):  # [nb, N] -> [nb, P, nt] with [l, p, m] = b[l, 128m+p]
        return b.reshape(nb, nt, P).transpose(0, 2, 1)

    wq_s = qkv_w[:, :, idx_q]
    wk_s = qkv_w[:, :, idx_k]
    wv_s = qkv_w[:, :, idx_v]
    # LN output biases are not applied on-device; fold them into the next
    # layer's biases. Block l>0 inputs x8 lack be2[l-1]; LN1 output lacks be1.
    dt64 = np.float64
    prev_be2 = np.concatenate(
        [np.zeros((1, D), np.float32), be2[:-1]], axis=0).astype(dt64)
    bq_eff = qkv_b[:, idx_q] + np.einsum(
        "ld,ldf->lf", prev_be2, wq_s.astype(dt64)).astype(np.float32)
    bk_eff = qkv_b[:, idx_k] + np.einsum(
        "ld,ldf->lf", prev_be2, wk_s.astype(dt64)).astype(np.float32)
    bv_eff = qkv_b[:, idx_v] + np.einsum(
        "ld,ldf->lf", prev_be2, wv_s.astype(dt64)).astype(np.float32)
    fcb_eff = fc_b + np.einsum("ld,ldf->lf", bv_eff.astype(dt64),
                               fc_w.astype(dt64)).astype(np.float32)
    b1_eff = b1 + np.einsum("ld,ldf->lf", be1.astype(dt64),
                            w1.astype(dt64)).astype(np.float32)
    biases = np.concatenate([
        btile(bq_eff * WS, 4), btile(bk_eff * WS, 4),
        btile(fcb_eff * WS, 4), btile(b2 * WS, 4), btile(g1, 4),
        btile(be1, 4), btile(g2, 4), btile(be2, 4), btile(b1_eff * WS, 8),
        btile(be2 * WS, 4)], axis=2)

    def w8(a):
        return np.ascontiguousarray(a * WS).astype(f8t)

    common = {
        "wq": w8(wq_s),
        "wk": w8(wk_s),
        "wv": w8(wv_s),
        "fcw": w8(fc_w),
        "w1": np.ascontiguousarray(w1 * WS).astype(np.float16),
        "w2": np.ascontiguousarray(w2 * WS).astype(np.float16),
        "biases": np.ascontiguousarray(biases),
    }
    X = np.asarray(inputs["X"], dtype=np.float32)
    in_maps = []
    for c in range(N_CORES):
        b, r = c // 4, c % 4
        xT = np.ascontiguousarray(X[b, TOK * r:TOK * (r + 1), :].T) * WS
        in_maps.append({"xT": xT, **common})
    return in_maps


def get_nc(nb=NB, reps=1):
    key = (nb, reps)
    if key not in _CACHE:
        _CACHE[key] = _build(nb, reps)
    return _CACHE[key]


def kernel(**inputs):
    from concourse.bass_utils import run_bass_kernel_spmd

    nb = NB
    nc = get_nc(nb)
    in_maps = _host_prep(inputs, nb)
    res = run_bass_kernel_spmd(nc, in_maps, list(range(N_CORES)))
    Y = np.zeros((B, S, D), dtype=np.float32)
    for c in range(N_CORES):
        b, r = c // 4, c % 4
        Y[b, TOK * r:TOK * (r + 1), :] = res.results[c]["yT"].T
    return Y
